# revision 14
# baseline (speedup 1.0000x reference)
"""Trainium2 Bass kernel for a top-2 MoE layer (8 experts), expert-parallel
across 8 NeuronCores.

Math (per reference):
    logits = x @ router_w                    # [S, E] fp32
    top2 vals/idx; gates = softmax(top2)     # [S, 2]
    out = sum_e gate_e * (silu(x@w1[e]) * (x@w3[e])) @ w2[e]

Distribution strategy (expert-parallel, host-side dispatch): the router GEMM
is 0.05% of total FLOPs, so the host computes it exactly in fp32 and
dispatches (token, expert) pairs to the 8 cores. Each core's program is a
pure streaming SwiGLU FFN over a fixed schedule of 4 weight "segments"
(9+8+8+8 = 33 tiles of 128 tokens = 4224 token slots); the host bin-packs
each expert's token list into the 32 (core, segment) slots so every real
(token, expert) pair is computed exactly once (3.1% padding). Expert weights
for each (core, segment) are uploaded per slot; w1/w3 stay SBUF-resident for
a whole segment while w2 streams per output tile. Gates are applied on the
host during the final gather-combine (y is linear in w2's output, so the
device returns ungated per-pair outputs in a transposed [d, token] layout
and the host does out[t] = g0*y[pos0[t]] + g1*y[pos1[t]]).

The device pipeline per block (up to 512 tokens): DMA x-block -> 22x
(8 matmuls w1 + 8 matmuls w3 -> PSUM; Silu on Scalar; mult on Vector ->
s_all bf16) -> GEMM2 (8 output tiles x 22 matmuls, w2 streamed) -> DMA out.
GEMM2 for block b is emitted after GEMM1/3 of block b+1 within a segment so
the PE never waits on the Vector engine; it is flushed at segment end so the
next segment's w1/w3 DMAs overlap the last two GEMM2s.
"""

import os
import sys

for _p in ("/opt/trn_rl_repo",):
    if _p not in sys.path and os.path.isdir(_p):
        sys.path.insert(0, _p)

from contextlib import ExitStack
from dataclasses import dataclass

import numpy as np
import ml_dtypes

from concourse import bacc, bass, mybir
import concourse.tile as tile

F32 = mybir.dt.float32
BF16 = mybir.dt.bfloat16


@dataclass(frozen=True)
class Cfg:
    S: int = 16384      # tokens
    D: int = 1024       # d_model
    H: int = 2816       # hidden
    E: int = 8          # experts == n_cores
    SEG_TILES: tuple = (9, 8, 8, 8)  # 128-token tiles per weight segment

    @property
    def DC(self):
        return self.D // 128

    @property
    def HC(self):
        return self.H // 128

    @property
    def TPC(self):
        return 128 * sum(self.SEG_TILES)  # token slots per core


REAL = Cfg()


def _blocks_of(ntiles):
    """Split a segment of `ntiles` 128-token tiles into matmul blocks of at
    most 4 tiles (PSUM bank = 512 fp32 cols). The first block is as large as
    possible so the PE outpaces the segment's weight-load DMA; the remainder
    is split evenly (e.g. 9 -> [4, 3, 2], 8 -> [4, 4])."""
    if ntiles <= 4:
        return [128 * ntiles]
    rest = ntiles - 4
    nb = -(-rest // 4)
    base, rem = divmod(rest, nb)
    sizes = [4] + [base + (1 if i < rem else 0) for i in range(nb)]
    return [128 * s for s in sizes]


def build_program(cfg: Cfg, debug: bool = False):
    c = cfg
    NSEG = len(c.SEG_TILES)
    seg_blocks = [_blocks_of(t) for t in c.SEG_TILES]

    nc = bacc.Bacc(
        "TRN2", target_bir_lowering=False, debug=debug, num_devices=c.E
    )

    xin = nc.dram_tensor(
        "xin", [128, c.DC * c.TPC], BF16, kind="ExternalInput"
    ).ap()
    w13_d = [
        nc.dram_tensor(
            f"w13_s{i}", [128, c.HC * 2 * c.DC * 128], BF16, kind="ExternalInput"
        ).ap()
        for i in range(NSEG)
    ]
    w2_d = [
        nc.dram_tensor(
            f"w2_s{i}", [128, c.DC * c.HC * 128], BF16, kind="ExternalInput"
        ).ap()
        for i in range(NSEG)
    ]
    yt_out = nc.dram_tensor(
        "yt", [128, c.DC * c.TPC], BF16, kind="ExternalOutput"
    ).ap()

    with ExitStack() as ctx:
        tc = ctx.enter_context(tile.TileContext(nc))

        wpool = ctx.enter_context(tc.tile_pool(name="w13", bufs=1))
        w2pool = ctx.enter_context(tc.tile_pool(name="w2s", bufs=2))
        xpool = ctx.enter_context(tc.tile_pool(name="xg", bufs=2))
        spool = ctx.enter_context(tc.tile_pool(name="sall", bufs=3))
        ypool = ctx.enter_context(tc.tile_pool(name="yt", bufs=2))
        apool = ctx.enter_context(tc.tile_pool(name="act", bufs=2))
        psum = ctx.enter_context(tc.tile_pool(name="psum", bufs=2, space="PSUM"))

        def emit_g2(si, s_t, goff, tb, preloaded=None):
            yt_t = ypool.tile([128, c.DC, tb], BF16, tag="yt")
            for d in range(c.DC):
                if preloaded is not None and d < len(preloaded):
                    w2d = preloaded[d]
                else:
                    w2d = w2pool.tile([128, c.HC * 128], BF16, tag="w2d")
                    nc.sync.dma_start(
                        out=w2d[:],
                        in_=w2_d[si][:, d * c.HC * 128 : (d + 1) * c.HC * 128],
                    )
                p2 = psum.tile([128, tb], F32, tag="p2")
                for hc in range(c.HC):
                    nc.tensor.matmul(
                        out=p2[:],
                        lhsT=w2d[:, hc * 128 : (hc + 1) * 128],
                        rhs=s_t[:, hc, :],
                        start=(hc == 0),
                        stop=(hc == c.HC - 1),
                    )
                nc.vector.tensor_copy(out=yt_t[:, d, :], in_=p2[:])
                # write out per d-tile so the final DMA isn't on the tail
                nc.sync.dma_start(
                    out=yt_out[:, goff * c.DC + d * tb : goff * c.DC + (d + 1) * tb],
                    in_=yt_t[:, d, :],
                )

        PRE_W13 = 2  # segment-0 w13 tiles pre-issued before compute starts

        def dma_w13(eng, si, hc, t):
            eng.dma_start(
                out=t[:],
                in_=w13_d[si][
                    :, hc * 2 * c.DC * 128 : (hc + 1) * 2 * c.DC * 128
                ],
            )

        def dma_xin(eng, goff, tb, xg):
            half = c.DC // 2
            eng.dma_start(
                out=xg[:, :half, :],
                in_=xin[:, goff * c.DC : goff * c.DC + half * tb],
            )
            eng.dma_start(
                out=xg[:, half:, :],
                in_=xin[:, goff * c.DC + half * tb : (goff + tb) * c.DC],
            )

        def emit_g13_hc(xg, s_t, tb, hc, w13sb, after_silu=None):
            w1h = w13sb[hc][:, : c.DC * 128]
            w3h = w13sb[hc][:, c.DC * 128 :]
            p1 = psum.tile([128, tb], F32, tag="p1")
            p3 = psum.tile([128, tb], F32, tag="p3")
            for k in range(c.DC):
                nc.tensor.matmul(
                    out=p1[:],
                    lhsT=w1h[:, k * 128 : (k + 1) * 128],
                    rhs=xg[:, k, :],
                    start=(k == 0),
                    stop=(k == c.DC - 1),
                )
            for k in range(c.DC):
                nc.tensor.matmul(
                    out=p3[:],
                    lhsT=w3h[:, k * 128 : (k + 1) * 128],
                    rhs=xg[:, k, :],
                    start=(k == 0),
                    stop=(k == c.DC - 1),
                )
            silu_t = apool.tile([128, tb], F32, tag="silu")
            nc.scalar.activation(
                silu_t[:], p1[:], mybir.ActivationFunctionType.Silu
            )
            if after_silu is not None:
                after_silu()
            nc.vector.tensor_tensor(
                out=s_t[:, hc, :], in0=silu_t[:], in1=p3[:],
                op=mybir.AluOpType.mult,
            )

        goff = 0
        for si in range(NSEG):
            # (re)load this segment's w1/w3 into resident SBUF tiles. For
            # si>0 the WAR deps on the previous segment's last reads stagger
            # these DMAs to the PE's per-hc cadence, and all 22 are issued
            # up front on the sync queue.
            w13sb = [
                wpool.tile(
                    [128, 2 * c.DC * 128], BF16, tag=f"w13_{hc}",
                    name=f"w13sb_s{si}_{hc}",
                )
                for hc in range(c.HC)
            ]
            if si > 0:
                for hc in range(c.HC):
                    dma_w13(nc.sync, si, hc, w13sb[hc])

            pending = []
            if si == 0:
                # Startup: there is no WAR gating for segment 0, and the DMA
                # rings process all transfers queued at t=0 breadth-first
                # (and ramp slowly), so a flood delays even the first tile by
                # the whole load time. Pre-issue only 2 w13 tiles + both x
                # blocks, release the rest from the scalar (silu) stream
                # which advances at the PE's pace, and fuse the first two
                # blocks' GEMM1/3 at the hc level so the PE does ~7us of work
                # per weight tile and stays ahead of the cold DMA rings.
                tb0, tb1 = seg_blocks[0][0], seg_blocks[0][1]
                xg0 = xpool.tile([128, c.DC, tb0], BF16, tag="xg")
                xg1 = xpool.tile([128, c.DC, tb1], BF16, tag="xg")
                for hc in range(PRE_W13):
                    dma_w13(nc.sync, si, hc, w13sb[hc])
                dma_xin(nc.sync, goff, tb0, xg0)
                dma_xin(nc.sync, goff + tb0, tb1, xg1)
                s0 = spool.tile([128, c.HC, tb0], BF16, tag="s")
                s1 = spool.tile([128, c.HC, tb1], BF16, tag="s")
                w2_pre = []

                def releases(hc):
                    def go():
                        if hc + PRE_W13 < c.HC:
                            dma_w13(
                                nc.scalar, 0, hc + PRE_W13, w13sb[hc + PRE_W13]
                            )
                        if hc in (c.HC - 3, c.HC - 2, c.HC - 1):
                            w2d = w2pool.tile(
                                [128, c.HC * 128], BF16, tag="w2d"
                            )
                            d = hc - (c.HC - 3)
                            nc.scalar.dma_start(
                                out=w2d[:],
                                in_=w2_d[0][
                                    :, d * c.HC * 128 : (d + 1) * c.HC * 128
                                ],
                            )
                            w2_pre.append(w2d)

                    return go

                for hc in range(c.HC):
                    emit_g13_hc(xg0, s0, tb0, hc, w13sb)
                    emit_g13_hc(xg1, s1, tb1, hc, w13sb, after_silu=releases(hc))
                pending = [
                    (0, s0, goff, tb0, w2_pre),
                    (0, s1, goff + tb0, tb1, None),
                ]
                goff += tb0 + tb1
                rest = list(enumerate(seg_blocks[si]))[2:]
            else:
                rest = list(enumerate(seg_blocks[si]))

            for bi, tb in rest:
                xg = xpool.tile([128, c.DC, tb], BF16, tag="xg")
                dma_xin(nc.sync, goff, tb, xg)
                s_t = spool.tile([128, c.HC, tb], BF16, tag="s")
                if pending and len(pending) >= 2:
                    emit_g2(*pending.pop(0))
                for hc in range(c.HC):
                    emit_g13_hc(xg, s_t, tb, hc, w13sb)
                pending.append((si, s_t, goff, tb, None))
                goff += tb
            # flush at segment end so the next segment's w13 DMAs hide
            # behind the trailing GEMM2s instead of stalling the PE
            for p in pending:
                emit_g2(*p)

    nc.compile()
    return nc


# ---------------- host-side routing, dispatch and combine ----------------


def _plan_bins(tiles, seg_tiles, n_cores):
    """Assign each expert's tile count to (core, segment) slots.

    Returns a list `slot_expert[core][seg] = expert id` and per-expert list
    of slot capacities in assignment order, or None if infeasible."""
    from collections import Counter
    from functools import lru_cache

    avail = Counter()
    for s in seg_tiles:
        avail[s] += n_cores
    sizes = sorted(avail, reverse=True)
    order = sorted(range(len(tiles)), key=lambda e: -tiles[e])
    assign = {}

    def options(need, av):
        # multisets of bin sizes covering `need`, sorted by (waste, nbins)
        res = []

        def rec(i, used, total):
            if total >= need:
                res.append((total - need, sum(used.values()), dict(used)))
                return
            if i == len(sizes):
                return
            s = sizes[i]
            maxn = min(av[s], -(-need // s))
            for n in range(maxn, -1, -1):
                if n:
                    used[s] = n
                rec(i + 1, used, total + n * s)
                used.pop(s, None)

        rec(0, {}, 0)
        res.sort(key=lambda r: (r[0], r[1]))
        return res

    def bt(i, av):
        if i == len(order):
            return True
        e = order[i]
        for waste, nb, used in options(tiles[e], av):
            av2 = av.copy()
            ok = all(av2[s] >= n for s, n in used.items())
            if not ok:
                continue
            for s, n in used.items():
                av2[s] -= n
            assign[e] = used
            if bt(i + 1, av2):
                return True
            del assign[e]
        return False

    if not bt(0, avail):
        return None

    # materialize slots: slot list in (core, seg) order with capacities
    slot_expert = [[None] * len(seg_tiles) for _ in range(n_cores)]
    free = {s: [] for s in sizes}
    for core in range(n_cores):
        for seg, s in enumerate(seg_tiles):
            free[s].append((core, seg))
    expert_slots = {}
    for e in order:
        sl = []
        for s in sorted(assign[e], reverse=True):
            for _ in range(assign[e][s]):
                core, seg = free[s].pop(0)
                slot_expert[core][seg] = e
                sl.append((core, seg, s))
        expert_slots[e] = sl
    # unused slots -> expert 0 with zero tokens
    for core in range(n_cores):
        for seg in range(len(seg_tiles)):
            if slot_expert[core][seg] is None:
                slot_expert[core][seg] = 0
    return slot_expert, expert_slots


def _host_route(cfg, x, router_w):
    c = cfg
    xf = np.ascontiguousarray(
        np.asarray(x, dtype=np.float32).reshape(c.S, c.D)
    )
    logits = xf @ np.asarray(router_w, dtype=np.float32)  # [S, E] fp32
    idx = np.argsort(-logits, axis=1, kind="stable")[:, :2]  # ties: low idx
    v = np.take_along_axis(logits, idx, axis=1)
    v = v - v.max(axis=1, keepdims=True)
    ev = np.exp(v)
    gates = ev / ev.sum(axis=1, keepdims=True)  # [S, 2] fp32
    return xf, idx, gates


def _prep(cfg, xf, idx):
    """Build per-core xin arrays + slot bookkeeping from routing decisions."""
    c = cfg
    NSEG = len(c.SEG_TILES)
    counts = np.bincount(idx.reshape(-1), minlength=c.E)
    tiles = [-(-int(cn) // 128) for cn in counts]

    plan = _plan_bins(tiles, c.SEG_TILES, c.E)
    if plan is None:
        raise RuntimeError(f"bin planning failed for counts {counts}")
    slot_expert, expert_slots = plan

    # expert pair lists: (token, rank) sorted by token then rank
    pair_t = {}
    pair_r = {}
    for e in range(c.E):
        t_arr, r_arr = np.nonzero(idx == e)
        pair_t[e] = t_arr.astype(np.int64)
        pair_r[e] = r_arr.astype(np.int64)

    seg_off = np.cumsum([0] + [128 * t for t in c.SEG_TILES])[:-1]
    # token slot table per core and position map (token, rank) -> global row
    tok_core = np.full((c.E, c.TPC), -1, dtype=np.int64)
    pos = np.full((c.S, 2), -1, dtype=np.int64)
    for e in range(c.E):
        off = 0
        for (core, seg, s) in expert_slots[e]:
            cap = 128 * s
            n = min(cap, len(pair_t[e]) - off)
            if n <= 0:
                continue
            rows = seg_off[seg] + np.arange(n)
            tok_core[core, rows] = pair_t[e][off : off + n]
            pos[pair_t[e][off : off + n], pair_r[e][off : off + n]] = (
                core * c.TPC + rows
            )
            off += n
        assert off >= len(pair_t[e]), f"expert {e} tokens unassigned"
    assert (pos >= 0).all(), "unassigned (token, rank) pair"

    # per-core xin in block layout [128, (b, k, t)]
    xbf = xf.astype(ml_dtypes.bfloat16)
    blocks = []
    goff = 0
    for st in c.SEG_TILES:
        for tb in _blocks_of(st):
            blocks.append((goff, tb))
            goff += tb
    xins = []
    for core in range(c.E):
        toks = tok_core[core]
        g = xbf[np.clip(toks, 0, None)]
        g[toks < 0] = 0
        parts = []
        for (boff, tb) in blocks:
            blk = g[boff : boff + tb]  # [tb, D]
            parts.append(
                np.ascontiguousarray(
                    blk.reshape(tb, c.DC, 128).transpose(2, 1, 0)
                ).reshape(128, c.DC * tb)
            )
        xins.append(np.ascontiguousarray(np.concatenate(parts, axis=1)))
    return slot_expert, pos, xins, blocks


def _prep_weights(cfg, w1, w3, w2):
    c = cfg
    W13, W2 = [], []
    for e in range(c.E):
        w1e = np.asarray(w1[e], dtype=np.float32).astype(ml_dtypes.bfloat16)
        w3e = np.asarray(w3[e], dtype=np.float32).astype(ml_dtypes.bfloat16)
        w2e = np.asarray(w2[e], dtype=np.float32).astype(ml_dtypes.bfloat16)
        w1te = (
            w1e.reshape(c.DC, 128, c.HC, 128)
            .transpose(1, 2, 0, 3)
            .reshape(128, c.HC * c.DC * 128)
        )
        w3te = (
            w3e.reshape(c.DC, 128, c.HC, 128)
            .transpose(1, 2, 0, 3)
            .reshape(128, c.HC * c.DC * 128)
        )
        w13te = np.ascontiguousarray(
            np.stack([w1te, w3te], axis=1)
            .reshape(128, 2, c.HC, c.DC * 128)
            .transpose(0, 2, 1, 3)
            .reshape(128, c.HC * 2 * c.DC * 128)
        )
        w2te = np.ascontiguousarray(
            w2e.reshape(c.HC, 128, c.DC, 128)
            .transpose(1, 2, 0, 3)
            .reshape(128, c.DC * c.HC * 128)
        )
        W13.append(w13te)
        W2.append(w2te)
    return W13, W2


def _combine(cfg, results, pos, gates, blocks):
    c = cfg
    ys = []
    for core in range(c.E):
        yt = np.asarray(results[core]["yt"])  # [128, DC*TPC] bf16
        parts = []
        col = 0
        for (boff, tb) in blocks:
            blk = yt[:, col : col + c.DC * tb].reshape(128, c.DC, tb)
            parts.append(
                blk.transpose(2, 1, 0).reshape(tb, c.D).astype(np.float32)
            )
            col += c.DC * tb
        ys.append(np.concatenate(parts, axis=0))
    y_all = np.concatenate(ys, axis=0)  # [E*TPC, D] fp32 ungated
    out = (
        gates[:, 0:1] * y_all[pos[:, 0]] + gates[:, 1:2] * y_all[pos[:, 1]]
    )
    return out


_PROGRAM_CACHE = {}


def _get_program(cfg: Cfg):
    if cfg not in _PROGRAM_CACHE:
        _PROGRAM_CACHE[cfg] = build_program(cfg, debug=False)
    return _PROGRAM_CACHE[cfg]


def _install_trace_shims():
    """The agent image's antenv lacks axon_hooks; recreate it from the
    boot package's ctypes NTFF driver so trace=True works under axon."""
    import types

    try:
        import antenv
        from antenv.axon_hooks import get_axon_ntff_profile_hook  # noqa: F401

        have = True
    except ImportError:
        have = False
    if not have:
        try:
            import antenv
            from trn_agent_boot.trn_boot import _ntff_profile_via_ctypes

            hook = _ntff_profile_via_ctypes("/opt/axon/libaxon_pjrt.so")
            mod = types.ModuleType("antenv.axon_hooks")
            mod.get_axon_ntff_profile_hook = lambda: hook
            mod.set_axon_ntff_profile_hook = lambda h: None
            sys.modules["antenv.axon_hooks"] = mod
            antenv.axon_hooks = mod
        except Exception as e:
            print(f"trace shim failed ({e}); tracing disabled")
            return False
    from concourse import bass_utils as _bu

    _orig_upload = _bu.upload_artifacts

    def _safe_upload(tmpdir):
        try:
            return _orig_upload(tmpdir)
        except Exception as e:
            return f"upload-skipped({e.__class__.__name__}):{tmpdir}"

    _bu.upload_artifacts = _safe_upload
    return True


def run(cfg: Cfg, x, router_w, w1, w3, w2, trace=False):
    from concourse.bass_utils import run_bass_kernel_spmd

    if trace and not _install_trace_shims():
        trace = False

    c = cfg
    xf, idx, gates = _host_route(c, x, router_w)
    counts = np.bincount(idx.reshape(-1), minlength=c.E)
    tiles = [-(-int(cn) // 128) for cn in counts]
    # grow segments if the planned capacity is infeasible (recompiles)
    for _ in range(16):
        if _plan_bins(tiles, c.SEG_TILES, c.E) is not None:
            break
        st = list(c.SEG_TILES)
        st[0] += 1
        c = Cfg(SEG_TILES=tuple(st))
    else:
        raise RuntimeError(f"no feasible bin plan for counts {counts}")

    slot_expert, pos, xins, blocks = _prep(c, xf, idx)
    W13, W2 = _prep_weights(c, w1, w3, w2)

    in_maps = []
    for core in range(c.E):
        m = {"xin": xins[core]}
        for seg in range(len(c.SEG_TILES)):
            e = slot_expert[core][seg]
            m[f"w13_s{seg}"] = W13[e]
            m[f"w2_s{seg}"] = W2[e]
        in_maps.append(m)

    nc = _get_program(c)
    res = run_bass_kernel_spmd(
        nc, in_maps, core_ids=list(range(c.E)), trace=trace
    )
    out = _combine(c, res.results, pos, gates, blocks)
    return out, res


def kernel(x, router_w, w1, w3, w2):
    out, _ = run(REAL, x, router_w, w1, w3, w2, trace=False)
    return out.reshape(np.asarray(x).shape).astype(np.float32)


if __name__ == "__main__":
    nc = build_program(REAL)
    print("built ok")


# revision 17
# speedup vs baseline: 1.0284x; 1.0284x over previous
"""Trainium2 Bass kernel for a top-2 MoE layer (8 experts), expert-parallel
across 8 NeuronCores.

Math (per reference):
    logits = x @ router_w                    # [S, E] fp32
    top2 vals/idx; gates = softmax(top2)     # [S, 2]
    out = sum_e gate_e * (silu(x@w1[e]) * (x@w3[e])) @ w2[e]

Distribution strategy (expert-parallel, host-side dispatch): the router GEMM
is 0.05% of total FLOPs, so the host computes it exactly in fp32 and
dispatches (token, expert) pairs to the 8 cores. Each core's program is a
pure streaming SwiGLU FFN over a fixed schedule of 4 weight "segments"
(9+8+8+8 = 33 tiles of 128 tokens = 4224 token slots); the host bin-packs
each expert's token list into the 32 (core, segment) slots so every real
(token, expert) pair is computed exactly once (3.1% padding). Expert weights
for each (core, segment) are uploaded per slot; w1/w3 stay SBUF-resident for
a whole segment while w2 streams per output tile. Gates are applied on the
host during the final gather-combine (y is linear in w2's output, so the
device returns ungated per-pair outputs in a transposed [d, token] layout
and the host does out[t] = g0*y[pos0[t]] + g1*y[pos1[t]]).

The device pipeline per block (up to 512 tokens): DMA x-block -> 22x
(8 matmuls w1 + 8 matmuls w3 -> PSUM; Silu on Scalar; mult on Vector ->
s_all bf16) -> GEMM2 (8 output tiles x 22 matmuls, w2 streamed) -> DMA out.
GEMM2 for block b is emitted after GEMM1/3 of block b+1 within a segment so
the PE never waits on the Vector engine; it is flushed at segment end so the
next segment's w1/w3 DMAs overlap the last two GEMM2s.
"""

import os
import sys

for _p in ("/opt/trn_rl_repo",):
    if _p not in sys.path and os.path.isdir(_p):
        sys.path.insert(0, _p)

from contextlib import ExitStack
from dataclasses import dataclass

import numpy as np
import ml_dtypes

from concourse import bacc, bass, mybir
import concourse.tile as tile

F32 = mybir.dt.float32
BF16 = mybir.dt.bfloat16


@dataclass(frozen=True)
class Cfg:
    S: int = 16384      # tokens
    D: int = 1024       # d_model
    H: int = 2816       # hidden
    E: int = 8          # experts == n_cores
    SEG_TILES: tuple = (9, 8, 8, 8)  # 128-token tiles per weight segment

    @property
    def DC(self):
        return self.D // 128

    @property
    def HC(self):
        return self.H // 128

    @property
    def TPC(self):
        return 128 * sum(self.SEG_TILES)  # token slots per core


REAL = Cfg()


def _blocks_of(ntiles):
    """Split a segment of `ntiles` 128-token tiles into matmul blocks of at
    most 4 tiles (PSUM bank = 512 fp32 cols). The first block is as large as
    possible so the PE outpaces the segment's weight-load DMA; the remainder
    is split evenly (e.g. 9 -> [4, 3, 2], 8 -> [4, 4])."""
    if ntiles <= 4:
        return [128 * ntiles]
    rest = ntiles - 4
    nb = -(-rest // 4)
    base, rem = divmod(rest, nb)
    sizes = [4] + [base + (1 if i < rem else 0) for i in range(nb)]
    return [128 * s for s in sizes]


def build_program(cfg: Cfg, debug: bool = False):
    c = cfg
    NSEG = len(c.SEG_TILES)
    seg_blocks = [_blocks_of(t) for t in c.SEG_TILES]

    nc = bacc.Bacc(
        "TRN2", target_bir_lowering=False, debug=debug, num_devices=c.E
    )

    xin = nc.dram_tensor(
        "xin", [128, c.DC * c.TPC], BF16, kind="ExternalInput"
    ).ap()
    w13_d = [
        nc.dram_tensor(
            f"w13_s{i}", [128, c.HC * 2 * c.DC * 128], BF16, kind="ExternalInput"
        ).ap()
        for i in range(NSEG)
    ]
    w2_d = [
        nc.dram_tensor(
            f"w2_s{i}", [128, c.DC * c.HC * 128], BF16, kind="ExternalInput"
        ).ap()
        for i in range(NSEG)
    ]
    yt_out = nc.dram_tensor(
        "yt", [128, c.DC * c.TPC], BF16, kind="ExternalOutput"
    ).ap()

    with ExitStack() as ctx:
        tc = ctx.enter_context(tile.TileContext(nc))

        wpool = ctx.enter_context(tc.tile_pool(name="w13", bufs=1))
        w2pool = ctx.enter_context(tc.tile_pool(name="w2s", bufs=3))
        xpool = ctx.enter_context(tc.tile_pool(name="xg", bufs=2))
        spool = ctx.enter_context(tc.tile_pool(name="sall", bufs=2))
        ypool = ctx.enter_context(tc.tile_pool(name="yt", bufs=2))
        apool = ctx.enter_context(tc.tile_pool(name="act", bufs=2))
        psum = ctx.enter_context(tc.tile_pool(name="psum", bufs=2, space="PSUM"))

        def emit_g2(si, s_t, goff, tb, preloaded=None):
            yt_t = ypool.tile([128, c.DC, tb], BF16, tag="yt")
            for d in range(c.DC):
                if preloaded is not None and d < len(preloaded):
                    w2d = preloaded[d]
                else:
                    w2d = w2pool.tile([128, c.HC * 128], BF16, tag="w2d")
                    nc.sync.dma_start(
                        out=w2d[:],
                        in_=w2_d[si][:, d * c.HC * 128 : (d + 1) * c.HC * 128],
                    )
                p2 = psum.tile([128, tb], F32, tag="p2")
                for hc in range(c.HC):
                    nc.tensor.matmul(
                        out=p2[:],
                        lhsT=w2d[:, hc * 128 : (hc + 1) * 128],
                        rhs=s_t[:, hc, :],
                        start=(hc == 0),
                        stop=(hc == c.HC - 1),
                    )
                nc.vector.tensor_copy(out=yt_t[:, d, :], in_=p2[:])
                # write out per d-tile so the final DMA isn't on the tail
                nc.sync.dma_start(
                    out=yt_out[:, goff * c.DC + d * tb : goff * c.DC + (d + 1) * tb],
                    in_=yt_t[:, d, :],
                )

        PRE_W13 = 4  # segment-0 w13 tiles pre-issued before compute starts

        def dma_w13(eng, si, hc, t):
            eng.dma_start(
                out=t[:],
                in_=w13_d[si][
                    :, hc * 2 * c.DC * 128 : (hc + 1) * 2 * c.DC * 128
                ],
            )

        def dma_xin(eng, goff, tb, xg):
            half = c.DC // 2
            eng.dma_start(
                out=xg[:, :half, :],
                in_=xin[:, goff * c.DC : goff * c.DC + half * tb],
            )
            eng.dma_start(
                out=xg[:, half:, :],
                in_=xin[:, goff * c.DC + half * tb : (goff + tb) * c.DC],
            )

        def emit_g13_hc(xg, s_t, tb, hc, w13sb, after_silu=None):
            w1h = w13sb[hc][:, : c.DC * 128]
            w3h = w13sb[hc][:, c.DC * 128 :]
            p1 = psum.tile([128, tb], F32, tag="p1")
            p3 = psum.tile([128, tb], F32, tag="p3")
            for k in range(c.DC):
                nc.tensor.matmul(
                    out=p1[:],
                    lhsT=w1h[:, k * 128 : (k + 1) * 128],
                    rhs=xg[:, k, :],
                    start=(k == 0),
                    stop=(k == c.DC - 1),
                )
            for k in range(c.DC):
                nc.tensor.matmul(
                    out=p3[:],
                    lhsT=w3h[:, k * 128 : (k + 1) * 128],
                    rhs=xg[:, k, :],
                    start=(k == 0),
                    stop=(k == c.DC - 1),
                )
            silu_t = apool.tile([128, tb], F32, tag="silu")
            nc.scalar.activation(
                silu_t[:], p1[:], mybir.ActivationFunctionType.Silu
            )
            if after_silu is not None:
                after_silu()
            nc.vector.tensor_tensor(
                out=s_t[:, hc, :], in0=silu_t[:], in1=p3[:],
                op=mybir.AluOpType.mult,
            )

        goff = 0
        for si in range(NSEG):
            # (re)load this segment's w1/w3 into resident SBUF tiles. For
            # si>0 the WAR deps on the previous segment's last reads stagger
            # these DMAs to the PE's per-hc cadence. For si==0 there is no
            # such gating, and the DMA rings process all transfers queued at
            # t=0 breadth-first, so a flood delays even the first tile by
            # the whole load time; instead pre-issue only the first few
            # tiles and release the rest from the scalar (silu) stream,
            # which advances at the PE's pace.
            w13sb = [
                wpool.tile(
                    [128, 2 * c.DC * 128], BF16, tag=f"w13_{hc}",
                    name=f"w13sb_s{si}_{hc}",
                )
                for hc in range(c.HC)
            ]
            for hc in range(PRE_W13 if si == 0 else c.HC):
                dma_w13(nc.sync, si, hc, w13sb[hc])

            pending = []
            xg_pre = {}
            if si == 0:
                # pre-allocate blocks 0/1 tiles so block 1's xin DMA can be
                # released from block 0's scalar stream (t=0 flood control)
                for bi in (0, 1):
                    xg_pre[bi] = xpool.tile(
                        [128, c.DC, seg_blocks[0][bi]], BF16, tag="xg",
                        name=f"xg_pre{bi}",
                    )
                xg_b1_todo = (
                    goff + seg_blocks[0][0], seg_blocks[0][1], xg_pre[1]
                )
            for bi, tb in enumerate(seg_blocks[si]):
                first = si == 0 and bi == 0
                xg = xg_pre.get(bi) if si == 0 else None
                if xg is None:
                    xg = xpool.tile([128, c.DC, tb], BF16, tag="xg")
                if not (si == 0 and bi == 1):
                    dma_xin(nc.sync, goff, tb, xg)
                s_t = spool.tile([128, c.HC, tb], BF16, tag="s")
                w2_pre = [] if first else None

                def releases(hc, w2_pre=w2_pre, first=first):
                    if not first:
                        return None

                    def go():
                        # scalar-stream-released DMAs, paced by PE progress
                        if hc + PRE_W13 < c.HC:
                            dma_w13(
                                nc.scalar, 0, hc + PRE_W13, w13sb[hc + PRE_W13]
                            )
                        if hc == 0:
                            dma_xin(nc.scalar, *xg_b1_todo)
                        if hc in (c.HC - 3, c.HC - 2, c.HC - 1):
                            w2d = w2pool.tile(
                                [128, c.HC * 128], BF16, tag="w2d"
                            )
                            d = hc - (c.HC - 3)
                            nc.scalar.dma_start(
                                out=w2d[:],
                                in_=w2_d[0][
                                    :, d * c.HC * 128 : (d + 1) * c.HC * 128
                                ],
                            )
                            w2_pre.append(w2d)

                    return go

                for hc in range(c.HC):
                    emit_g13_hc(xg, s_t, tb, hc, w13sb, after_silu=releases(hc))
                pending.append((si, s_t, goff, tb, w2_pre))
                if len(pending) > 1:
                    emit_g2(*pending.pop(0))
                goff += tb
            # flush at segment end so the next segment's w13 DMAs hide
            # behind the trailing GEMM2s instead of stalling the PE
            for p in pending:
                emit_g2(*p)

    nc.compile()
    return nc


# ---------------- host-side routing, dispatch and combine ----------------


def _plan_bins(tiles, seg_tiles, n_cores):
    """Assign each expert's tile count to (core, segment) slots.

    Returns a list `slot_expert[core][seg] = expert id` and per-expert list
    of slot capacities in assignment order, or None if infeasible."""
    from collections import Counter
    from functools import lru_cache

    avail = Counter()
    for s in seg_tiles:
        avail[s] += n_cores
    sizes = sorted(avail, reverse=True)
    order = sorted(range(len(tiles)), key=lambda e: -tiles[e])
    assign = {}

    def options(need, av):
        # multisets of bin sizes covering `need`, sorted by (waste, nbins)
        res = []

        def rec(i, used, total):
            if total >= need:
                res.append((total - need, sum(used.values()), dict(used)))
                return
            if i == len(sizes):
                return
            s = sizes[i]
            maxn = min(av[s], -(-need // s))
            for n in range(maxn, -1, -1):
                if n:
                    used[s] = n
                rec(i + 1, used, total + n * s)
                used.pop(s, None)

        rec(0, {}, 0)
        res.sort(key=lambda r: (r[0], r[1]))
        return res

    def bt(i, av):
        if i == len(order):
            return True
        e = order[i]
        for waste, nb, used in options(tiles[e], av):
            av2 = av.copy()
            ok = all(av2[s] >= n for s, n in used.items())
            if not ok:
                continue
            for s, n in used.items():
                av2[s] -= n
            assign[e] = used
            if bt(i + 1, av2):
                return True
            del assign[e]
        return False

    if not bt(0, avail):
        return None

    # materialize slots: slot list in (core, seg) order with capacities
    slot_expert = [[None] * len(seg_tiles) for _ in range(n_cores)]
    free = {s: [] for s in sizes}
    for core in range(n_cores):
        for seg, s in enumerate(seg_tiles):
            free[s].append((core, seg))
    expert_slots = {}
    for e in order:
        sl = []
        for s in sorted(assign[e], reverse=True):
            for _ in range(assign[e][s]):
                core, seg = free[s].pop(0)
                slot_expert[core][seg] = e
                sl.append((core, seg, s))
        expert_slots[e] = sl
    # unused slots -> expert 0 with zero tokens
    for core in range(n_cores):
        for seg in range(len(seg_tiles)):
            if slot_expert[core][seg] is None:
                slot_expert[core][seg] = 0
    return slot_expert, expert_slots


def _host_route(cfg, x, router_w):
    c = cfg
    xf = np.ascontiguousarray(
        np.asarray(x, dtype=np.float32).reshape(c.S, c.D)
    )
    logits = xf @ np.asarray(router_w, dtype=np.float32)  # [S, E] fp32
    idx = np.argsort(-logits, axis=1, kind="stable")[:, :2]  # ties: low idx
    v = np.take_along_axis(logits, idx, axis=1)
    v = v - v.max(axis=1, keepdims=True)
    ev = np.exp(v)
    gates = ev / ev.sum(axis=1, keepdims=True)  # [S, 2] fp32
    return xf, idx, gates


def _prep(cfg, xf, idx):
    """Build per-core xin arrays + slot bookkeeping from routing decisions."""
    c = cfg
    NSEG = len(c.SEG_TILES)
    counts = np.bincount(idx.reshape(-1), minlength=c.E)
    tiles = [-(-int(cn) // 128) for cn in counts]

    plan = _plan_bins(tiles, c.SEG_TILES, c.E)
    if plan is None:
        raise RuntimeError(f"bin planning failed for counts {counts}")
    slot_expert, expert_slots = plan

    # expert pair lists: (token, rank) sorted by token then rank
    pair_t = {}
    pair_r = {}
    for e in range(c.E):
        t_arr, r_arr = np.nonzero(idx == e)
        pair_t[e] = t_arr.astype(np.int64)
        pair_r[e] = r_arr.astype(np.int64)

    seg_off = np.cumsum([0] + [128 * t for t in c.SEG_TILES])[:-1]
    # token slot table per core and position map (token, rank) -> global row
    tok_core = np.full((c.E, c.TPC), -1, dtype=np.int64)
    pos = np.full((c.S, 2), -1, dtype=np.int64)
    for e in range(c.E):
        off = 0
        for (core, seg, s) in expert_slots[e]:
            cap = 128 * s
            n = min(cap, len(pair_t[e]) - off)
            if n <= 0:
                continue
            rows = seg_off[seg] + np.arange(n)
            tok_core[core, rows] = pair_t[e][off : off + n]
            pos[pair_t[e][off : off + n], pair_r[e][off : off + n]] = (
                core * c.TPC + rows
            )
            off += n
        assert off >= len(pair_t[e]), f"expert {e} tokens unassigned"
    assert (pos >= 0).all(), "unassigned (token, rank) pair"

    # per-core xin in block layout [128, (b, k, t)]
    xbf = xf.astype(ml_dtypes.bfloat16)
    blocks = []
    goff = 0
    for st in c.SEG_TILES:
        for tb in _blocks_of(st):
            blocks.append((goff, tb))
            goff += tb
    xins = []
    for core in range(c.E):
        toks = tok_core[core]
        g = xbf[np.clip(toks, 0, None)]
        g[toks < 0] = 0
        parts = []
        for (boff, tb) in blocks:
            blk = g[boff : boff + tb]  # [tb, D]
            parts.append(
                np.ascontiguousarray(
                    blk.reshape(tb, c.DC, 128).transpose(2, 1, 0)
                ).reshape(128, c.DC * tb)
            )
        xins.append(np.ascontiguousarray(np.concatenate(parts, axis=1)))
    return slot_expert, pos, xins, blocks


def _prep_weights(cfg, w1, w3, w2):
    c = cfg
    W13, W2 = [], []
    for e in range(c.E):
        w1e = np.asarray(w1[e], dtype=np.float32).astype(ml_dtypes.bfloat16)
        w3e = np.asarray(w3[e], dtype=np.float32).astype(ml_dtypes.bfloat16)
        w2e = np.asarray(w2[e], dtype=np.float32).astype(ml_dtypes.bfloat16)
        w1te = (
            w1e.reshape(c.DC, 128, c.HC, 128)
            .transpose(1, 2, 0, 3)
            .reshape(128, c.HC * c.DC * 128)
        )
        w3te = (
            w3e.reshape(c.DC, 128, c.HC, 128)
            .transpose(1, 2, 0, 3)
            .reshape(128, c.HC * c.DC * 128)
        )
        w13te = np.ascontiguousarray(
            np.stack([w1te, w3te], axis=1)
            .reshape(128, 2, c.HC, c.DC * 128)
            .transpose(0, 2, 1, 3)
            .reshape(128, c.HC * 2 * c.DC * 128)
        )
        w2te = np.ascontiguousarray(
            w2e.reshape(c.HC, 128, c.DC, 128)
            .transpose(1, 2, 0, 3)
            .reshape(128, c.DC * c.HC * 128)
        )
        W13.append(w13te)
        W2.append(w2te)
    return W13, W2


def _combine(cfg, results, pos, gates, blocks):
    c = cfg
    ys = []
    for core in range(c.E):
        yt = np.asarray(results[core]["yt"])  # [128, DC*TPC] bf16
        parts = []
        col = 0
        for (boff, tb) in blocks:
            blk = yt[:, col : col + c.DC * tb].reshape(128, c.DC, tb)
            parts.append(
                blk.transpose(2, 1, 0).reshape(tb, c.D).astype(np.float32)
            )
            col += c.DC * tb
        ys.append(np.concatenate(parts, axis=0))
    y_all = np.concatenate(ys, axis=0)  # [E*TPC, D] fp32 ungated
    out = (
        gates[:, 0:1] * y_all[pos[:, 0]] + gates[:, 1:2] * y_all[pos[:, 1]]
    )
    return out


_PROGRAM_CACHE = {}


def _get_program(cfg: Cfg):
    if cfg not in _PROGRAM_CACHE:
        _PROGRAM_CACHE[cfg] = build_program(cfg, debug=False)
    return _PROGRAM_CACHE[cfg]


def _install_trace_shims():
    """The agent image's antenv lacks axon_hooks; recreate it from the
    boot package's ctypes NTFF driver so trace=True works under axon."""
    import types

    try:
        import antenv
        from antenv.axon_hooks import get_axon_ntff_profile_hook  # noqa: F401

        have = True
    except ImportError:
        have = False
    if not have:
        try:
            import antenv
            from trn_agent_boot.trn_boot import _ntff_profile_via_ctypes

            hook = _ntff_profile_via_ctypes("/opt/axon/libaxon_pjrt.so")
            mod = types.ModuleType("antenv.axon_hooks")
            mod.get_axon_ntff_profile_hook = lambda: hook
            mod.set_axon_ntff_profile_hook = lambda h: None
            sys.modules["antenv.axon_hooks"] = mod
            antenv.axon_hooks = mod
        except Exception as e:
            print(f"trace shim failed ({e}); tracing disabled")
            return False
    from concourse import bass_utils as _bu

    _orig_upload = _bu.upload_artifacts

    def _safe_upload(tmpdir):
        try:
            return _orig_upload(tmpdir)
        except Exception as e:
            return f"upload-skipped({e.__class__.__name__}):{tmpdir}"

    _bu.upload_artifacts = _safe_upload
    return True


def run(cfg: Cfg, x, router_w, w1, w3, w2, trace=False):
    from concourse.bass_utils import run_bass_kernel_spmd

    if trace and not _install_trace_shims():
        trace = False

    c = cfg
    xf, idx, gates = _host_route(c, x, router_w)
    counts = np.bincount(idx.reshape(-1), minlength=c.E)
    tiles = [-(-int(cn) // 128) for cn in counts]
    # grow segments if the planned capacity is infeasible (recompiles)
    for _ in range(16):
        if _plan_bins(tiles, c.SEG_TILES, c.E) is not None:
            break
        st = list(c.SEG_TILES)
        st[0] += 1
        c = Cfg(SEG_TILES=tuple(st))
    else:
        raise RuntimeError(f"no feasible bin plan for counts {counts}")

    slot_expert, pos, xins, blocks = _prep(c, xf, idx)
    W13, W2 = _prep_weights(c, w1, w3, w2)

    in_maps = []
    for core in range(c.E):
        m = {"xin": xins[core]}
        for seg in range(len(c.SEG_TILES)):
            e = slot_expert[core][seg]
            m[f"w13_s{seg}"] = W13[e]
            m[f"w2_s{seg}"] = W2[e]
        in_maps.append(m)

    nc = _get_program(c)
    res = run_bass_kernel_spmd(
        nc, in_maps, core_ids=list(range(c.E)), trace=trace
    )
    out = _combine(c, res.results, pos, gates, blocks)
    return out, res


def kernel(x, router_w, w1, w3, w2):
    out, _ = run(REAL, x, router_w, w1, w3, w2, trace=False)
    return out.reshape(np.asarray(x).shape).astype(np.float32)


if __name__ == "__main__":
    nc = build_program(REAL)
    print("built ok")


# revision 19
# speedup vs baseline: 1.0491x; 1.0202x over previous
"""Trainium2 Bass kernel for a top-2 MoE layer (8 experts), expert-parallel
across 8 NeuronCores.

Math (per reference):
    logits = x @ router_w                    # [S, E] fp32
    top2 vals/idx; gates = softmax(top2)     # [S, 2]
    out = sum_e gate_e * (silu(x@w1[e]) * (x@w3[e])) @ w2[e]

Distribution strategy (expert-parallel, host-side dispatch): the router GEMM
is 0.05% of total FLOPs, so the host computes it exactly in fp32 and
dispatches (token, expert) pairs to the 8 cores. Each core's program is a
pure streaming SwiGLU FFN over a fixed schedule of 4 weight "segments"
(9+8+8+8 = 33 tiles of 128 tokens = 4224 token slots); the host bin-packs
each expert's token list into the 32 (core, segment) slots so every real
(token, expert) pair is computed exactly once (3.1% padding). Expert weights
for each (core, segment) are uploaded per slot; w1/w3 stay SBUF-resident for
a whole segment while w2 streams per output tile. Gates are applied on the
host during the final gather-combine (y is linear in w2's output, so the
device returns ungated per-pair outputs in a transposed [d, token] layout
and the host does out[t] = g0*y[pos0[t]] + g1*y[pos1[t]]).

The device pipeline per block (up to 512 tokens): DMA x-block -> 22x
(8 matmuls w1 + 8 matmuls w3 -> PSUM; Silu on Scalar; mult on Vector ->
s_all bf16) -> GEMM2 (8 output tiles x 22 matmuls, w2 streamed) -> DMA out.
GEMM2 for block b is emitted after GEMM1/3 of block b+1 within a segment so
the PE never waits on the Vector engine; it is flushed at segment end so the
next segment's w1/w3 DMAs overlap the last two GEMM2s.
"""

import os
import sys

for _p in ("/opt/trn_rl_repo",):
    if _p not in sys.path and os.path.isdir(_p):
        sys.path.insert(0, _p)

from contextlib import ExitStack
from dataclasses import dataclass

import numpy as np
import ml_dtypes

from concourse import bacc, bass, mybir
import concourse.tile as tile

F32 = mybir.dt.float32
BF16 = mybir.dt.bfloat16
UNIT = 64  # dispatch granularity in tokens


@dataclass(frozen=True)
class Cfg:
    S: int = 16384      # tokens
    D: int = 1024       # d_model
    H: int = 2816       # hidden
    E: int = 8          # experts == n_cores
    SEG_UNITS: tuple = (21, 16, 15, 13)  # 64-token units per weight segment

    @property
    def DC(self):
        return self.D // 128

    @property
    def HC(self):
        return self.H // 128

    @property
    def TPC(self):
        return UNIT * sum(self.SEG_UNITS)  # token slots per core


REAL = Cfg()


def _blocks_of(nunits):
    """Split a segment of `nunits` 64-token units into matmul blocks of at
    most 8 units = 512 tokens (PSUM bank = 512 fp32 cols). The first block is
    as large as possible so the PE outpaces the segment's weight-load DMA;
    the remainder is split evenly (e.g. 21 -> [512, 448, 384])."""
    if nunits <= 8:
        return [UNIT * nunits]
    rest = nunits - 8
    nb = -(-rest // 8)
    base, rem = divmod(rest, nb)
    sizes = [8] + [base + (1 if i < rem else 0) for i in range(nb)]
    return [UNIT * s for s in sizes]


def build_program(cfg: Cfg, debug: bool = False):
    c = cfg
    NSEG = len(c.SEG_UNITS)
    seg_blocks = [_blocks_of(u) for u in c.SEG_UNITS]

    nc = bacc.Bacc(
        "TRN2", target_bir_lowering=False, debug=debug, num_devices=c.E
    )

    xin = nc.dram_tensor(
        "xin", [128, c.DC * c.TPC], BF16, kind="ExternalInput"
    ).ap()
    w13_d = [
        nc.dram_tensor(
            f"w13_s{i}", [128, c.HC * 2 * c.DC * 128], BF16, kind="ExternalInput"
        ).ap()
        for i in range(NSEG)
    ]
    w2_d = [
        nc.dram_tensor(
            f"w2_s{i}", [128, c.DC * c.HC * 128], BF16, kind="ExternalInput"
        ).ap()
        for i in range(NSEG)
    ]
    yt_out = nc.dram_tensor(
        "yt", [128, c.DC * c.TPC], BF16, kind="ExternalOutput"
    ).ap()

    with ExitStack() as ctx:
        tc = ctx.enter_context(tile.TileContext(nc))

        wpool = ctx.enter_context(tc.tile_pool(name="w13", bufs=1))
        w2pool = ctx.enter_context(tc.tile_pool(name="w2s", bufs=3))
        xpool = ctx.enter_context(tc.tile_pool(name="xg", bufs=2))
        spool = ctx.enter_context(tc.tile_pool(name="sall", bufs=2))
        ypool = ctx.enter_context(tc.tile_pool(name="yt", bufs=2))
        apool = ctx.enter_context(tc.tile_pool(name="act", bufs=2))
        psum = ctx.enter_context(tc.tile_pool(name="psum", bufs=2, space="PSUM"))

        def emit_g2(si, s_t, goff, tb, preloaded=None):
            yt_t = ypool.tile([128, c.DC, tb], BF16, tag="yt")
            for d in range(c.DC):
                if preloaded is not None and d < len(preloaded):
                    w2d = preloaded[d]
                else:
                    w2d = w2pool.tile([128, c.HC * 128], BF16, tag="w2d")
                    nc.sync.dma_start(
                        out=w2d[:],
                        in_=w2_d[si][:, d * c.HC * 128 : (d + 1) * c.HC * 128],
                    )
                p2 = psum.tile([128, tb], F32, tag="p2")
                for hc in range(c.HC):
                    nc.tensor.matmul(
                        out=p2[:],
                        lhsT=w2d[:, hc * 128 : (hc + 1) * 128],
                        rhs=s_t[:, hc, :],
                        start=(hc == 0),
                        stop=(hc == c.HC - 1),
                    )
                nc.vector.tensor_copy(out=yt_t[:, d, :], in_=p2[:])
                # write out per d-tile so the final DMA isn't on the tail
                nc.sync.dma_start(
                    out=yt_out[:, goff * c.DC + d * tb : goff * c.DC + (d + 1) * tb],
                    in_=yt_t[:, d, :],
                )

        PRE_W13 = 4  # segment-0 w13 tiles pre-issued before compute starts

        def dma_w13(eng, si, hc, t):
            eng.dma_start(
                out=t[:],
                in_=w13_d[si][
                    :, hc * 2 * c.DC * 128 : (hc + 1) * 2 * c.DC * 128
                ],
            )

        def dma_xin(eng, goff, tb, xg):
            half = c.DC // 2
            eng.dma_start(
                out=xg[:, :half, :],
                in_=xin[:, goff * c.DC : goff * c.DC + half * tb],
            )
            eng.dma_start(
                out=xg[:, half:, :],
                in_=xin[:, goff * c.DC + half * tb : (goff + tb) * c.DC],
            )

        def emit_g13_hc(xg, s_t, tb, hc, w13sb, after_silu=None):
            w1h = w13sb[hc][:, : c.DC * 128]
            w3h = w13sb[hc][:, c.DC * 128 :]
            p1 = psum.tile([128, tb], F32, tag="p1")
            p3 = psum.tile([128, tb], F32, tag="p3")
            for k in range(c.DC):
                nc.tensor.matmul(
                    out=p1[:],
                    lhsT=w1h[:, k * 128 : (k + 1) * 128],
                    rhs=xg[:, k, :],
                    start=(k == 0),
                    stop=(k == c.DC - 1),
                )
            for k in range(c.DC):
                nc.tensor.matmul(
                    out=p3[:],
                    lhsT=w3h[:, k * 128 : (k + 1) * 128],
                    rhs=xg[:, k, :],
                    start=(k == 0),
                    stop=(k == c.DC - 1),
                )
            silu_t = apool.tile([128, tb], F32, tag="silu")
            nc.scalar.activation(
                silu_t[:], p1[:], mybir.ActivationFunctionType.Silu
            )
            if after_silu is not None:
                after_silu()
            nc.vector.tensor_tensor(
                out=s_t[:, hc, :], in0=silu_t[:], in1=p3[:],
                op=mybir.AluOpType.mult,
            )

        goff = 0
        for si in range(NSEG):
            # (re)load this segment's w1/w3 into resident SBUF tiles. For
            # si>0 the WAR deps on the previous segment's last reads stagger
            # these DMAs to the PE's per-hc cadence. For si==0 there is no
            # such gating, and the DMA rings process all transfers queued at
            # t=0 breadth-first, so a flood delays even the first tile by
            # the whole load time; instead pre-issue only the first few
            # tiles and release the rest from the scalar (silu) stream,
            # which advances at the PE's pace.
            w13sb = [
                wpool.tile(
                    [128, 2 * c.DC * 128], BF16, tag=f"w13_{hc}",
                    name=f"w13sb_s{si}_{hc}",
                )
                for hc in range(c.HC)
            ]
            for hc in range(PRE_W13 if si == 0 else c.HC):
                dma_w13(nc.sync, si, hc, w13sb[hc])

            pending = []
            xg_pre = {}
            if si == 0:
                # pre-allocate blocks 0/1 tiles so block 1's xin DMA can be
                # released from block 0's scalar stream (t=0 flood control)
                for bi in (0, 1):
                    xg_pre[bi] = xpool.tile(
                        [128, c.DC, seg_blocks[0][bi]], BF16, tag="xg",
                        name=f"xg_pre{bi}",
                    )
                xg_b1_todo = (
                    goff + seg_blocks[0][0], seg_blocks[0][1], xg_pre[1]
                )
            for bi, tb in enumerate(seg_blocks[si]):
                first = si == 0 and bi == 0
                xg = xg_pre.get(bi) if si == 0 else None
                if xg is None:
                    xg = xpool.tile([128, c.DC, tb], BF16, tag="xg")
                if not (si == 0 and bi == 1):
                    dma_xin(nc.sync, goff, tb, xg)
                s_t = spool.tile([128, c.HC, tb], BF16, tag="s")
                w2_pre = [] if first else None

                def releases(hc, w2_pre=w2_pre, first=first):
                    if not first:
                        return None

                    def go():
                        # scalar-stream-released DMAs, paced by PE progress
                        if hc + PRE_W13 < c.HC:
                            dma_w13(
                                nc.scalar, 0, hc + PRE_W13, w13sb[hc + PRE_W13]
                            )
                        if hc == 0:
                            dma_xin(nc.scalar, *xg_b1_todo)
                        if hc in (c.HC - 3, c.HC - 2, c.HC - 1):
                            w2d = w2pool.tile(
                                [128, c.HC * 128], BF16, tag="w2d"
                            )
                            d = hc - (c.HC - 3)
                            nc.scalar.dma_start(
                                out=w2d[:],
                                in_=w2_d[0][
                                    :, d * c.HC * 128 : (d + 1) * c.HC * 128
                                ],
                            )
                            w2_pre.append(w2d)

                    return go

                for hc in range(c.HC):
                    emit_g13_hc(xg, s_t, tb, hc, w13sb, after_silu=releases(hc))
                pending.append((si, s_t, goff, tb, w2_pre))
                if len(pending) > 1:
                    emit_g2(*pending.pop(0))
                goff += tb
            # flush at segment end so the next segment's w13 DMAs hide
            # behind the trailing GEMM2s instead of stalling the PE
            for p in pending:
                emit_g2(*p)

    nc.compile()
    return nc


# ---------------- host-side routing, dispatch and combine ----------------


def _plan_bins(needs, seg_units, n_cores):
    """Assign each expert's unit count to (core, segment) slots.

    `needs` and `seg_units` are in UNIT-token units. Returns
    (slot_expert[core][seg] = expert id, expert_slots[e] = [(core, seg,
    size_units), ...]) or None if infeasible. Search is slack-pruned
    (total overshoot across experts is bounded by spare capacity) with
    memoized failure states."""
    sizes = sorted(set(seg_units), reverse=True)
    ns = len(sizes)
    avail0 = tuple(list(seg_units).count(s) * n_cores for s in sizes)
    order = sorted(range(len(needs)), key=lambda e: -needs[e])
    slack0 = sum(seg_units) * n_cores - sum(needs)
    if slack0 < 0:
        return None
    seen = set()
    assign = {}

    def options(need, av, slack):
        res = []

        def rec(i, used, total):
            if total >= need:
                if total - need <= slack:
                    res.append(
                        (total - need, tuple(used) + (0,) * (ns - len(used)))
                    )
                return
            if i == ns:
                return
            for n in range(min(av[i], -(-need // sizes[i])), -1, -1):
                rec(i + 1, used + [n], total + n * sizes[i])

        rec(0, [], 0)
        res.sort()
        return res

    def bt(i, av, slack):
        if i == len(order):
            return True
        key = (i, av, slack)
        if key in seen:
            return False
        e = order[i]
        for waste, used in options(needs[e], av, slack):
            assign[e] = used
            if bt(
                i + 1,
                tuple(av[j] - used[j] for j in range(ns)),
                slack - waste,
            ):
                return True
            del assign[e]
        seen.add(key)
        return False

    if not bt(0, avail0, slack0):
        return None

    # materialize slots: slot list in (core, seg) order with capacities
    slot_expert = [[None] * len(seg_units) for _ in range(n_cores)]
    free = {s: [] for s in sizes}
    for core in range(n_cores):
        for seg, s in enumerate(seg_units):
            free[s].append((core, seg))
    expert_slots = {}
    for e in order:
        sl = []
        for j, s in enumerate(sizes):
            for _ in range(assign[e][j]):
                core, seg = free[s].pop(0)
                slot_expert[core][seg] = e
                sl.append((core, seg, s))
        expert_slots[e] = sl
    # unused slots -> expert 0 with zero tokens
    for core in range(n_cores):
        for seg in range(len(seg_units)):
            if slot_expert[core][seg] is None:
                slot_expert[core][seg] = 0
    return slot_expert, expert_slots


def _host_route(cfg, x, router_w):
    c = cfg
    xf = np.ascontiguousarray(
        np.asarray(x, dtype=np.float32).reshape(c.S, c.D)
    )
    logits = xf @ np.asarray(router_w, dtype=np.float32)  # [S, E] fp32
    idx = np.argsort(-logits, axis=1, kind="stable")[:, :2]  # ties: low idx
    v = np.take_along_axis(logits, idx, axis=1)
    v = v - v.max(axis=1, keepdims=True)
    ev = np.exp(v)
    gates = ev / ev.sum(axis=1, keepdims=True)  # [S, 2] fp32
    return xf, idx, gates


def _prep(cfg, xf, idx):
    """Build per-core xin arrays + slot bookkeeping from routing decisions."""
    c = cfg
    NSEG = len(c.SEG_UNITS)
    counts = np.bincount(idx.reshape(-1), minlength=c.E)
    needs = [-(-int(cn) // UNIT) for cn in counts]

    plan = _plan_bins(needs, c.SEG_UNITS, c.E)
    if plan is None:
        raise RuntimeError(f"bin planning failed for counts {counts}")
    slot_expert, expert_slots = plan

    # expert pair lists: (token, rank) sorted by token then rank
    pair_t = {}
    pair_r = {}
    for e in range(c.E):
        t_arr, r_arr = np.nonzero(idx == e)
        pair_t[e] = t_arr.astype(np.int64)
        pair_r[e] = r_arr.astype(np.int64)

    seg_off = np.cumsum([0] + [UNIT * u for u in c.SEG_UNITS])[:-1]
    # token slot table per core and position map (token, rank) -> global row
    tok_core = np.full((c.E, c.TPC), -1, dtype=np.int64)
    pos = np.full((c.S, 2), -1, dtype=np.int64)
    for e in range(c.E):
        off = 0
        for (core, seg, s) in expert_slots[e]:
            cap = UNIT * s
            n = min(cap, len(pair_t[e]) - off)
            if n <= 0:
                continue
            rows = seg_off[seg] + np.arange(n)
            tok_core[core, rows] = pair_t[e][off : off + n]
            pos[pair_t[e][off : off + n], pair_r[e][off : off + n]] = (
                core * c.TPC + rows
            )
            off += n
        assert off >= len(pair_t[e]), f"expert {e} tokens unassigned"
    assert (pos >= 0).all(), "unassigned (token, rank) pair"

    # per-core xin in block layout [128, (b, k, t)]
    xbf = xf.astype(ml_dtypes.bfloat16)
    blocks = []
    goff = 0
    for st in c.SEG_UNITS:
        for tb in _blocks_of(st):
            blocks.append((goff, tb))
            goff += tb
    xins = []
    for core in range(c.E):
        toks = tok_core[core]
        g = xbf[np.clip(toks, 0, None)]
        g[toks < 0] = 0
        parts = []
        for (boff, tb) in blocks:
            blk = g[boff : boff + tb]  # [tb, D]
            parts.append(
                np.ascontiguousarray(
                    blk.reshape(tb, c.DC, 128).transpose(2, 1, 0)
                ).reshape(128, c.DC * tb)
            )
        xins.append(np.ascontiguousarray(np.concatenate(parts, axis=1)))
    return slot_expert, pos, xins, blocks


def _prep_weights(cfg, w1, w3, w2):
    c = cfg
    W13, W2 = [], []
    for e in range(c.E):
        w1e = np.asarray(w1[e], dtype=np.float32).astype(ml_dtypes.bfloat16)
        w3e = np.asarray(w3[e], dtype=np.float32).astype(ml_dtypes.bfloat16)
        w2e = np.asarray(w2[e], dtype=np.float32).astype(ml_dtypes.bfloat16)
        w1te = (
            w1e.reshape(c.DC, 128, c.HC, 128)
            .transpose(1, 2, 0, 3)
            .reshape(128, c.HC * c.DC * 128)
        )
        w3te = (
            w3e.reshape(c.DC, 128, c.HC, 128)
            .transpose(1, 2, 0, 3)
            .reshape(128, c.HC * c.DC * 128)
        )
        w13te = np.ascontiguousarray(
            np.stack([w1te, w3te], axis=1)
            .reshape(128, 2, c.HC, c.DC * 128)
            .transpose(0, 2, 1, 3)
            .reshape(128, c.HC * 2 * c.DC * 128)
        )
        w2te = np.ascontiguousarray(
            w2e.reshape(c.HC, 128, c.DC, 128)
            .transpose(1, 2, 0, 3)
            .reshape(128, c.DC * c.HC * 128)
        )
        W13.append(w13te)
        W2.append(w2te)
    return W13, W2


def _combine(cfg, results, pos, gates, blocks):
    c = cfg
    ys = []
    for core in range(c.E):
        yt = np.asarray(results[core]["yt"])  # [128, DC*TPC] bf16
        parts = []
        col = 0
        for (boff, tb) in blocks:
            blk = yt[:, col : col + c.DC * tb].reshape(128, c.DC, tb)
            parts.append(
                blk.transpose(2, 1, 0).reshape(tb, c.D).astype(np.float32)
            )
            col += c.DC * tb
        ys.append(np.concatenate(parts, axis=0))
    y_all = np.concatenate(ys, axis=0)  # [E*TPC, D] fp32 ungated
    out = (
        gates[:, 0:1] * y_all[pos[:, 0]] + gates[:, 1:2] * y_all[pos[:, 1]]
    )
    return out


_PROGRAM_CACHE = {}


def _get_program(cfg: Cfg):
    if cfg not in _PROGRAM_CACHE:
        _PROGRAM_CACHE[cfg] = build_program(cfg, debug=False)
    return _PROGRAM_CACHE[cfg]


def _install_trace_shims():
    """The agent image's antenv lacks axon_hooks; recreate it from the
    boot package's ctypes NTFF driver so trace=True works under axon."""
    import types

    try:
        import antenv
        from antenv.axon_hooks import get_axon_ntff_profile_hook  # noqa: F401

        have = True
    except ImportError:
        have = False
    if not have:
        try:
            import antenv
            from trn_agent_boot.trn_boot import _ntff_profile_via_ctypes

            hook = _ntff_profile_via_ctypes("/opt/axon/libaxon_pjrt.so")
            mod = types.ModuleType("antenv.axon_hooks")
            mod.get_axon_ntff_profile_hook = lambda: hook
            mod.set_axon_ntff_profile_hook = lambda h: None
            sys.modules["antenv.axon_hooks"] = mod
            antenv.axon_hooks = mod
        except Exception as e:
            print(f"trace shim failed ({e}); tracing disabled")
            return False
    from concourse import bass_utils as _bu

    _orig_upload = _bu.upload_artifacts

    def _safe_upload(tmpdir):
        try:
            return _orig_upload(tmpdir)
        except Exception as e:
            return f"upload-skipped({e.__class__.__name__}):{tmpdir}"

    _bu.upload_artifacts = _safe_upload
    return True


def run(cfg: Cfg, x, router_w, w1, w3, w2, trace=False):
    from concourse.bass_utils import run_bass_kernel_spmd

    if trace and not _install_trace_shims():
        trace = False

    c = cfg
    xf, idx, gates = _host_route(c, x, router_w)
    counts = np.bincount(idx.reshape(-1), minlength=c.E)
    needs = [-(-int(cn) // UNIT) for cn in counts]
    # grow segments if the planned capacity is infeasible (recompiles)
    for _ in range(64):
        if _plan_bins(needs, c.SEG_UNITS, c.E) is not None:
            break
        st = list(c.SEG_UNITS)
        st[0] += 1
        c = Cfg(SEG_UNITS=tuple(st))
    else:
        raise RuntimeError(f"no feasible bin plan for counts {counts}")

    slot_expert, pos, xins, blocks = _prep(c, xf, idx)
    W13, W2 = _prep_weights(c, w1, w3, w2)

    in_maps = []
    for core in range(c.E):
        m = {"xin": xins[core]}
        for seg in range(len(c.SEG_UNITS)):
            e = slot_expert[core][seg]
            m[f"w13_s{seg}"] = W13[e]
            m[f"w2_s{seg}"] = W2[e]
        in_maps.append(m)

    nc = _get_program(c)
    res = run_bass_kernel_spmd(
        nc, in_maps, core_ids=list(range(c.E)), trace=trace
    )
    out = _combine(c, res.results, pos, gates, blocks)
    return out, res


def kernel(x, router_w, w1, w3, w2):
    out, _ = run(REAL, x, router_w, w1, w3, w2, trace=False)
    return out.reshape(np.asarray(x).shape).astype(np.float32)


if __name__ == "__main__":
    nc = build_program(REAL)
    print("built ok")


# revision 20
# speedup vs baseline: 1.0496x; 1.0005x over previous
"""Trainium2 Bass kernel for a top-2 MoE layer (8 experts), expert-parallel
across 8 NeuronCores.

Math (per reference):
    logits = x @ router_w                    # [S, E] fp32
    top2 vals/idx; gates = softmax(top2)     # [S, 2]
    out = sum_e gate_e * (silu(x@w1[e]) * (x@w3[e])) @ w2[e]

Distribution strategy (expert-parallel, host-side dispatch): the router GEMM
is 0.05% of total FLOPs, so the host computes it exactly in fp32 and
dispatches (token, expert) pairs to the 8 cores. Each core's program is a
pure streaming SwiGLU FFN over a fixed schedule of 4 weight "segments"
(9+8+8+8 = 33 tiles of 128 tokens = 4224 token slots); the host bin-packs
each expert's token list into the 32 (core, segment) slots so every real
(token, expert) pair is computed exactly once (3.1% padding). Expert weights
for each (core, segment) are uploaded per slot; w1/w3 stay SBUF-resident for
a whole segment while w2 streams per output tile. Gates are applied on the
host during the final gather-combine (y is linear in w2's output, so the
device returns ungated per-pair outputs in a transposed [d, token] layout
and the host does out[t] = g0*y[pos0[t]] + g1*y[pos1[t]]).

The device pipeline per block (up to 512 tokens): DMA x-block -> 22x
(8 matmuls w1 + 8 matmuls w3 -> PSUM; Silu on Scalar; mult on Vector ->
s_all bf16) -> GEMM2 (8 output tiles x 22 matmuls, w2 streamed) -> DMA out.
GEMM2 for block b is emitted after GEMM1/3 of block b+1 within a segment so
the PE never waits on the Vector engine; it is flushed at segment end so the
next segment's w1/w3 DMAs overlap the last two GEMM2s.
"""

import os
import sys

for _p in ("/opt/trn_rl_repo",):
    if _p not in sys.path and os.path.isdir(_p):
        sys.path.insert(0, _p)

from contextlib import ExitStack
from dataclasses import dataclass

import numpy as np
import ml_dtypes

from concourse import bacc, bass, mybir
import concourse.tile as tile

F32 = mybir.dt.float32
BF16 = mybir.dt.bfloat16
UNIT = 64  # dispatch granularity in tokens


@dataclass(frozen=True)
class Cfg:
    S: int = 16384      # tokens
    D: int = 1024       # d_model
    H: int = 2816       # hidden
    E: int = 8          # experts == n_cores
    SEG_UNITS: tuple = (21, 16, 15, 13)  # 64-token units per weight segment

    @property
    def DC(self):
        return self.D // 128

    @property
    def HC(self):
        return self.H // 128

    @property
    def TPC(self):
        return UNIT * sum(self.SEG_UNITS)  # token slots per core


REAL = Cfg()


def _blocks_of(nunits):
    """Split a segment of `nunits` 64-token units into matmul blocks of at
    most 8 units = 512 tokens (PSUM bank = 512 fp32 cols). The first block is
    as large as possible so the PE outpaces the segment's weight-load DMA;
    the remainder is split evenly (e.g. 21 -> [512, 448, 384])."""
    if nunits <= 8:
        return [UNIT * nunits]
    rest = nunits - 8
    nb = -(-rest // 8)
    base, rem = divmod(rest, nb)
    sizes = [8] + [base + (1 if i < rem else 0) for i in range(nb)]
    return [UNIT * s for s in sizes]


def build_program(cfg: Cfg, debug: bool = False):
    c = cfg
    NSEG = len(c.SEG_UNITS)
    seg_blocks = [_blocks_of(u) for u in c.SEG_UNITS]

    nc = bacc.Bacc(
        "TRN2", target_bir_lowering=False, debug=debug, num_devices=c.E
    )

    xin = nc.dram_tensor(
        "xin", [128, c.DC * c.TPC], BF16, kind="ExternalInput"
    ).ap()
    w13_d = [
        nc.dram_tensor(
            f"w13_s{i}", [128, c.HC * 2 * c.DC * 128], BF16, kind="ExternalInput"
        ).ap()
        for i in range(NSEG)
    ]
    w2_d = [
        nc.dram_tensor(
            f"w2_s{i}", [128, c.DC * c.HC * 128], BF16, kind="ExternalInput"
        ).ap()
        for i in range(NSEG)
    ]
    yt_out = nc.dram_tensor(
        "yt", [128, c.DC * c.TPC], BF16, kind="ExternalOutput"
    ).ap()

    with ExitStack() as ctx:
        tc = ctx.enter_context(tile.TileContext(nc))

        wpool = ctx.enter_context(tc.tile_pool(name="w13", bufs=1))
        w2pool = ctx.enter_context(tc.tile_pool(name="w2s", bufs=3))
        xpool = ctx.enter_context(tc.tile_pool(name="xg", bufs=2))
        spool = ctx.enter_context(tc.tile_pool(name="sall", bufs=2))
        ypool = ctx.enter_context(tc.tile_pool(name="yt", bufs=2))
        apool = ctx.enter_context(tc.tile_pool(name="act", bufs=2))
        psum = ctx.enter_context(tc.tile_pool(name="psum", bufs=2, space="PSUM"))

        def emit_g2(si, s_t, goff, tb, preloaded=None):
            yt_t = ypool.tile([128, c.DC, tb], BF16, tag="yt")
            for d in range(c.DC):
                if preloaded is not None and d < len(preloaded):
                    w2d = preloaded[d]
                else:
                    w2d = w2pool.tile([128, c.HC * 128], BF16, tag="w2d")
                    nc.sync.dma_start(
                        out=w2d[:],
                        in_=w2_d[si][:, d * c.HC * 128 : (d + 1) * c.HC * 128],
                    )
                p2 = psum.tile([128, tb], F32, tag="p2")
                for hc in range(c.HC):
                    nc.tensor.matmul(
                        out=p2[:],
                        lhsT=w2d[:, hc * 128 : (hc + 1) * 128],
                        rhs=s_t[:, hc, :],
                        start=(hc == 0),
                        stop=(hc == c.HC - 1),
                    )
                nc.vector.tensor_copy(out=yt_t[:, d, :], in_=p2[:])
                # write out per d-tile so the final DMA isn't on the tail
                nc.sync.dma_start(
                    out=yt_out[:, goff * c.DC + d * tb : goff * c.DC + (d + 1) * tb],
                    in_=yt_t[:, d, :],
                )

        PRE_W13 = 3  # segment-0 w13 tiles pre-issued before compute starts

        def dma_w13(eng, si, hc, t):
            # two half-DMAs (w1 | w3): finer quanta smooth the cold-start
            # stagger, and the first matmuls only need the w1 half
            base = hc * 2 * c.DC * 128
            mid = base + c.DC * 128
            eng.dma_start(out=t[:, : c.DC * 128], in_=w13_d[si][:, base:mid])
            eng.dma_start(
                out=t[:, c.DC * 128 :],
                in_=w13_d[si][:, mid : base + 2 * c.DC * 128],
            )

        def dma_xin(eng, goff, tb, xg):
            half = c.DC // 2
            eng.dma_start(
                out=xg[:, :half, :],
                in_=xin[:, goff * c.DC : goff * c.DC + half * tb],
            )
            eng.dma_start(
                out=xg[:, half:, :],
                in_=xin[:, goff * c.DC + half * tb : (goff + tb) * c.DC],
            )

        def emit_g13_hc(xg, s_t, tb, hc, w13sb, after_silu=None):
            w1h = w13sb[hc][:, : c.DC * 128]
            w3h = w13sb[hc][:, c.DC * 128 :]
            p1 = psum.tile([128, tb], F32, tag="p1")
            p3 = psum.tile([128, tb], F32, tag="p3")
            for k in range(c.DC):
                nc.tensor.matmul(
                    out=p1[:],
                    lhsT=w1h[:, k * 128 : (k + 1) * 128],
                    rhs=xg[:, k, :],
                    start=(k == 0),
                    stop=(k == c.DC - 1),
                )
            for k in range(c.DC):
                nc.tensor.matmul(
                    out=p3[:],
                    lhsT=w3h[:, k * 128 : (k + 1) * 128],
                    rhs=xg[:, k, :],
                    start=(k == 0),
                    stop=(k == c.DC - 1),
                )
            silu_t = apool.tile([128, tb], F32, tag="silu")
            nc.scalar.activation(
                silu_t[:], p1[:], mybir.ActivationFunctionType.Silu
            )
            if after_silu is not None:
                after_silu()
            nc.vector.tensor_tensor(
                out=s_t[:, hc, :], in0=silu_t[:], in1=p3[:],
                op=mybir.AluOpType.mult,
            )

        goff = 0
        for si in range(NSEG):
            # (re)load this segment's w1/w3 into resident SBUF tiles. For
            # si>0 the WAR deps on the previous segment's last reads stagger
            # these DMAs to the PE's per-hc cadence. For si==0 there is no
            # such gating, and the DMA rings process all transfers queued at
            # t=0 breadth-first, so a flood delays even the first tile by
            # the whole load time; instead pre-issue only the first few
            # tiles and release the rest from the scalar (silu) stream,
            # which advances at the PE's pace.
            w13sb = [
                wpool.tile(
                    [128, 2 * c.DC * 128], BF16, tag=f"w13_{hc}",
                    name=f"w13sb_s{si}_{hc}",
                )
                for hc in range(c.HC)
            ]
            for hc in range(PRE_W13 if si == 0 else c.HC):
                dma_w13(nc.sync, si, hc, w13sb[hc])

            pending = []
            xg_pre = {}
            if si == 0:
                # pre-allocate blocks 0/1 tiles so block 1's xin DMA can be
                # released from block 0's scalar stream (t=0 flood control)
                for bi in (0, 1):
                    xg_pre[bi] = xpool.tile(
                        [128, c.DC, seg_blocks[0][bi]], BF16, tag="xg",
                        name=f"xg_pre{bi}",
                    )
                xg_b1_todo = (
                    goff + seg_blocks[0][0], seg_blocks[0][1], xg_pre[1]
                )
            for bi, tb in enumerate(seg_blocks[si]):
                first = si == 0 and bi == 0
                xg = xg_pre.get(bi) if si == 0 else None
                if xg is None:
                    xg = xpool.tile([128, c.DC, tb], BF16, tag="xg")
                if not (si == 0 and bi == 1):
                    dma_xin(nc.sync, goff, tb, xg)
                s_t = spool.tile([128, c.HC, tb], BF16, tag="s")
                w2_pre = [] if first else None

                def releases(hc, w2_pre=w2_pre, first=first):
                    if not first:
                        return None

                    def go():
                        # scalar-stream-released DMAs, paced by PE progress
                        if hc + PRE_W13 < c.HC:
                            dma_w13(
                                nc.scalar, 0, hc + PRE_W13, w13sb[hc + PRE_W13]
                            )
                        if hc == 10:
                            dma_xin(nc.scalar, *xg_b1_todo)
                        if hc in (c.HC - 3, c.HC - 2, c.HC - 1):
                            w2d = w2pool.tile(
                                [128, c.HC * 128], BF16, tag="w2d"
                            )
                            d = hc - (c.HC - 3)
                            nc.scalar.dma_start(
                                out=w2d[:],
                                in_=w2_d[0][
                                    :, d * c.HC * 128 : (d + 1) * c.HC * 128
                                ],
                            )
                            w2_pre.append(w2d)

                    return go

                for hc in range(c.HC):
                    emit_g13_hc(xg, s_t, tb, hc, w13sb, after_silu=releases(hc))
                pending.append((si, s_t, goff, tb, w2_pre))
                if len(pending) > 1:
                    emit_g2(*pending.pop(0))
                goff += tb
            # flush at segment end so the next segment's w13 DMAs hide
            # behind the trailing GEMM2s instead of stalling the PE
            for p in pending:
                emit_g2(*p)

    nc.compile()
    return nc


# ---------------- host-side routing, dispatch and combine ----------------


def _plan_bins(needs, seg_units, n_cores):
    """Assign each expert's unit count to (core, segment) slots.

    `needs` and `seg_units` are in UNIT-token units. Returns
    (slot_expert[core][seg] = expert id, expert_slots[e] = [(core, seg,
    size_units), ...]) or None if infeasible. Search is slack-pruned
    (total overshoot across experts is bounded by spare capacity) with
    memoized failure states."""
    sizes = sorted(set(seg_units), reverse=True)
    ns = len(sizes)
    avail0 = tuple(list(seg_units).count(s) * n_cores for s in sizes)
    order = sorted(range(len(needs)), key=lambda e: -needs[e])
    slack0 = sum(seg_units) * n_cores - sum(needs)
    if slack0 < 0:
        return None
    seen = set()
    assign = {}

    def options(need, av, slack):
        res = []

        def rec(i, used, total):
            if total >= need:
                if total - need <= slack:
                    res.append(
                        (total - need, tuple(used) + (0,) * (ns - len(used)))
                    )
                return
            if i == ns:
                return
            for n in range(min(av[i], -(-need // sizes[i])), -1, -1):
                rec(i + 1, used + [n], total + n * sizes[i])

        rec(0, [], 0)
        res.sort()
        return res

    def bt(i, av, slack):
        if i == len(order):
            return True
        key = (i, av, slack)
        if key in seen:
            return False
        e = order[i]
        for waste, used in options(needs[e], av, slack):
            assign[e] = used
            if bt(
                i + 1,
                tuple(av[j] - used[j] for j in range(ns)),
                slack - waste,
            ):
                return True
            del assign[e]
        seen.add(key)
        return False

    if not bt(0, avail0, slack0):
        return None

    # materialize slots: slot list in (core, seg) order with capacities
    slot_expert = [[None] * len(seg_units) for _ in range(n_cores)]
    free = {s: [] for s in sizes}
    for core in range(n_cores):
        for seg, s in enumerate(seg_units):
            free[s].append((core, seg))
    expert_slots = {}
    for e in order:
        sl = []
        for j, s in enumerate(sizes):
            for _ in range(assign[e][j]):
                core, seg = free[s].pop(0)
                slot_expert[core][seg] = e
                sl.append((core, seg, s))
        expert_slots[e] = sl
    # unused slots -> expert 0 with zero tokens
    for core in range(n_cores):
        for seg in range(len(seg_units)):
            if slot_expert[core][seg] is None:
                slot_expert[core][seg] = 0
    return slot_expert, expert_slots


def _host_route(cfg, x, router_w):
    c = cfg
    xf = np.ascontiguousarray(
        np.asarray(x, dtype=np.float32).reshape(c.S, c.D)
    )
    logits = xf @ np.asarray(router_w, dtype=np.float32)  # [S, E] fp32
    idx = np.argsort(-logits, axis=1, kind="stable")[:, :2]  # ties: low idx
    v = np.take_along_axis(logits, idx, axis=1)
    v = v - v.max(axis=1, keepdims=True)
    ev = np.exp(v)
    gates = ev / ev.sum(axis=1, keepdims=True)  # [S, 2] fp32
    return xf, idx, gates


def _prep(cfg, xf, idx):
    """Build per-core xin arrays + slot bookkeeping from routing decisions."""
    c = cfg
    NSEG = len(c.SEG_UNITS)
    counts = np.bincount(idx.reshape(-1), minlength=c.E)
    needs = [-(-int(cn) // UNIT) for cn in counts]

    plan = _plan_bins(needs, c.SEG_UNITS, c.E)
    if plan is None:
        raise RuntimeError(f"bin planning failed for counts {counts}")
    slot_expert, expert_slots = plan

    # expert pair lists: (token, rank) sorted by token then rank
    pair_t = {}
    pair_r = {}
    for e in range(c.E):
        t_arr, r_arr = np.nonzero(idx == e)
        pair_t[e] = t_arr.astype(np.int64)
        pair_r[e] = r_arr.astype(np.int64)

    seg_off = np.cumsum([0] + [UNIT * u for u in c.SEG_UNITS])[:-1]
    # token slot table per core and position map (token, rank) -> global row
    tok_core = np.full((c.E, c.TPC), -1, dtype=np.int64)
    pos = np.full((c.S, 2), -1, dtype=np.int64)
    for e in range(c.E):
        off = 0
        for (core, seg, s) in expert_slots[e]:
            cap = UNIT * s
            n = min(cap, len(pair_t[e]) - off)
            if n <= 0:
                continue
            rows = seg_off[seg] + np.arange(n)
            tok_core[core, rows] = pair_t[e][off : off + n]
            pos[pair_t[e][off : off + n], pair_r[e][off : off + n]] = (
                core * c.TPC + rows
            )
            off += n
        assert off >= len(pair_t[e]), f"expert {e} tokens unassigned"
    assert (pos >= 0).all(), "unassigned (token, rank) pair"

    # per-core xin in block layout [128, (b, k, t)]
    xbf = xf.astype(ml_dtypes.bfloat16)
    blocks = []
    goff = 0
    for st in c.SEG_UNITS:
        for tb in _blocks_of(st):
            blocks.append((goff, tb))
            goff += tb
    xins = []
    for core in range(c.E):
        toks = tok_core[core]
        g = xbf[np.clip(toks, 0, None)]
        g[toks < 0] = 0
        parts = []
        for (boff, tb) in blocks:
            blk = g[boff : boff + tb]  # [tb, D]
            parts.append(
                np.ascontiguousarray(
                    blk.reshape(tb, c.DC, 128).transpose(2, 1, 0)
                ).reshape(128, c.DC * tb)
            )
        xins.append(np.ascontiguousarray(np.concatenate(parts, axis=1)))
    return slot_expert, pos, xins, blocks


def _prep_weights(cfg, w1, w3, w2):
    c = cfg
    W13, W2 = [], []
    for e in range(c.E):
        w1e = np.asarray(w1[e], dtype=np.float32).astype(ml_dtypes.bfloat16)
        w3e = np.asarray(w3[e], dtype=np.float32).astype(ml_dtypes.bfloat16)
        w2e = np.asarray(w2[e], dtype=np.float32).astype(ml_dtypes.bfloat16)
        w1te = (
            w1e.reshape(c.DC, 128, c.HC, 128)
            .transpose(1, 2, 0, 3)
            .reshape(128, c.HC * c.DC * 128)
        )
        w3te = (
            w3e.reshape(c.DC, 128, c.HC, 128)
            .transpose(1, 2, 0, 3)
            .reshape(128, c.HC * c.DC * 128)
        )
        w13te = np.ascontiguousarray(
            np.stack([w1te, w3te], axis=1)
            .reshape(128, 2, c.HC, c.DC * 128)
            .transpose(0, 2, 1, 3)
            .reshape(128, c.HC * 2 * c.DC * 128)
        )
        w2te = np.ascontiguousarray(
            w2e.reshape(c.HC, 128, c.DC, 128)
            .transpose(1, 2, 0, 3)
            .reshape(128, c.DC * c.HC * 128)
        )
        W13.append(w13te)
        W2.append(w2te)
    return W13, W2


def _combine(cfg, results, pos, gates, blocks):
    c = cfg
    ys = []
    for core in range(c.E):
        yt = np.asarray(results[core]["yt"])  # [128, DC*TPC] bf16
        parts = []
        col = 0
        for (boff, tb) in blocks:
            blk = yt[:, col : col + c.DC * tb].reshape(128, c.DC, tb)
            parts.append(
                blk.transpose(2, 1, 0).reshape(tb, c.D).astype(np.float32)
            )
            col += c.DC * tb
        ys.append(np.concatenate(parts, axis=0))
    y_all = np.concatenate(ys, axis=0)  # [E*TPC, D] fp32 ungated
    out = (
        gates[:, 0:1] * y_all[pos[:, 0]] + gates[:, 1:2] * y_all[pos[:, 1]]
    )
    return out


_PROGRAM_CACHE = {}


def _get_program(cfg: Cfg):
    if cfg not in _PROGRAM_CACHE:
        _PROGRAM_CACHE[cfg] = build_program(cfg, debug=False)
    return _PROGRAM_CACHE[cfg]


def _install_trace_shims():
    """The agent image's antenv lacks axon_hooks; recreate it from the
    boot package's ctypes NTFF driver so trace=True works under axon."""
    import types

    try:
        import antenv
        from antenv.axon_hooks import get_axon_ntff_profile_hook  # noqa: F401

        have = True
    except ImportError:
        have = False
    if not have:
        try:
            import antenv
            from trn_agent_boot.trn_boot import _ntff_profile_via_ctypes

            hook = _ntff_profile_via_ctypes("/opt/axon/libaxon_pjrt.so")
            mod = types.ModuleType("antenv.axon_hooks")
            mod.get_axon_ntff_profile_hook = lambda: hook
            mod.set_axon_ntff_profile_hook = lambda h: None
            sys.modules["antenv.axon_hooks"] = mod
            antenv.axon_hooks = mod
        except Exception as e:
            print(f"trace shim failed ({e}); tracing disabled")
            return False
    from concourse import bass_utils as _bu

    _orig_upload = _bu.upload_artifacts

    def _safe_upload(tmpdir):
        try:
            return _orig_upload(tmpdir)
        except Exception as e:
            return f"upload-skipped({e.__class__.__name__}):{tmpdir}"

    _bu.upload_artifacts = _safe_upload
    return True


def run(cfg: Cfg, x, router_w, w1, w3, w2, trace=False):
    from concourse.bass_utils import run_bass_kernel_spmd

    if trace and not _install_trace_shims():
        trace = False

    c = cfg
    xf, idx, gates = _host_route(c, x, router_w)
    counts = np.bincount(idx.reshape(-1), minlength=c.E)
    needs = [-(-int(cn) // UNIT) for cn in counts]
    # grow segments if the planned capacity is infeasible (recompiles)
    for _ in range(64):
        if _plan_bins(needs, c.SEG_UNITS, c.E) is not None:
            break
        st = list(c.SEG_UNITS)
        st[0] += 1
        c = Cfg(SEG_UNITS=tuple(st))
    else:
        raise RuntimeError(f"no feasible bin plan for counts {counts}")

    slot_expert, pos, xins, blocks = _prep(c, xf, idx)
    W13, W2 = _prep_weights(c, w1, w3, w2)

    in_maps = []
    for core in range(c.E):
        m = {"xin": xins[core]}
        for seg in range(len(c.SEG_UNITS)):
            e = slot_expert[core][seg]
            m[f"w13_s{seg}"] = W13[e]
            m[f"w2_s{seg}"] = W2[e]
        in_maps.append(m)

    nc = _get_program(c)
    res = run_bass_kernel_spmd(
        nc, in_maps, core_ids=list(range(c.E)), trace=trace
    )
    out = _combine(c, res.results, pos, gates, blocks)
    return out, res


def kernel(x, router_w, w1, w3, w2):
    out, _ = run(REAL, x, router_w, w1, w3, w2, trace=False)
    return out.reshape(np.asarray(x).shape).astype(np.float32)


if __name__ == "__main__":
    nc = build_program(REAL)
    print("built ok")


# revision 22
# speedup vs baseline: 1.0550x; 1.0051x over previous
"""Trainium2 Bass kernel for a top-2 MoE layer (8 experts), expert-parallel
across 8 NeuronCores.

Math (per reference):
    logits = x @ router_w                    # [S, E] fp32
    top2 vals/idx; gates = softmax(top2)     # [S, 2]
    out = sum_e gate_e * (silu(x@w1[e]) * (x@w3[e])) @ w2[e]

Distribution strategy (expert-parallel, host-side dispatch): the router GEMM
is 0.05% of total FLOPs, so the host computes it exactly in fp32 and
dispatches (token, expert) pairs to the 8 cores. Each core's program is a
pure streaming SwiGLU FFN over a fixed schedule of 4 weight "segments"
(9+8+8+8 = 33 tiles of 128 tokens = 4224 token slots); the host bin-packs
each expert's token list into the 32 (core, segment) slots so every real
(token, expert) pair is computed exactly once (3.1% padding). Expert weights
for each (core, segment) are uploaded per slot; w1/w3 stay SBUF-resident for
a whole segment while w2 streams per output tile. Gates are applied on the
host during the final gather-combine (y is linear in w2's output, so the
device returns ungated per-pair outputs in a transposed [d, token] layout
and the host does out[t] = g0*y[pos0[t]] + g1*y[pos1[t]]).

The device pipeline per block (up to 512 tokens): DMA x-block -> 22x
(8 matmuls w1 + 8 matmuls w3 -> PSUM; Silu on Scalar; mult on Vector ->
s_all bf16) -> GEMM2 (8 output tiles x 22 matmuls, w2 streamed) -> DMA out.
GEMM2 for block b is emitted after GEMM1/3 of block b+1 within a segment so
the PE never waits on the Vector engine; it is flushed at segment end so the
next segment's w1/w3 DMAs overlap the last two GEMM2s.
"""

import os
import sys

for _p in ("/opt/trn_rl_repo",):
    if _p not in sys.path and os.path.isdir(_p):
        sys.path.insert(0, _p)

from contextlib import ExitStack
from dataclasses import dataclass

import numpy as np
import ml_dtypes

from concourse import bacc, bass, mybir
import concourse.tile as tile

F32 = mybir.dt.float32
BF16 = mybir.dt.bfloat16
UNIT = 32  # dispatch granularity in tokens


@dataclass(frozen=True)
class Cfg:
    S: int = 16384      # tokens
    D: int = 1024       # d_model
    H: int = 2816       # hidden
    E: int = 8          # experts == n_cores
    SEG_UNITS: tuple = (48, 46, 21, 14)  # 32-token units per weight segment

    @property
    def DC(self):
        return self.D // 128

    @property
    def HC(self):
        return self.H // 128

    @property
    def TPC(self):
        return UNIT * sum(self.SEG_UNITS)  # token slots per core


REAL = Cfg()


BMAX = 512 // UNIT  # max units per matmul block (PSUM bank = 512 fp32)


def _even_split(n, cap):
    nb = -(-n // cap)
    base, rem = divmod(n, nb)
    return [base + (1 if i < rem else 0) for i in range(nb)]


def _blocks_of(nunits):
    """Split a segment of `nunits` UNIT-token units into matmul blocks of
    at most 512 tokens. Prefer a max-size first block (so the PE outpaces
    the segment's weight-load DMA at cold start) unless that leaves a runt
    block; blocks under ~290 tokens lose matmul efficiency."""
    if nunits <= BMAX:
        return [UNIT * nunits]
    first_big = [BMAX] + _even_split(nunits - BMAX, BMAX)
    if min(first_big) * UNIT >= 288:
        return [UNIT * s for s in first_big]
    return [UNIT * s for s in _even_split(nunits, BMAX)]


def build_program(cfg: Cfg, debug: bool = False):
    c = cfg
    NSEG = len(c.SEG_UNITS)
    seg_blocks = [_blocks_of(u) for u in c.SEG_UNITS]

    nc = bacc.Bacc(
        "TRN2", target_bir_lowering=False, debug=debug, num_devices=c.E
    )

    xin = nc.dram_tensor(
        "xin", [128, c.DC * c.TPC], BF16, kind="ExternalInput"
    ).ap()
    w13_d = [
        nc.dram_tensor(
            f"w13_s{i}", [128, c.HC * 2 * c.DC * 128], BF16, kind="ExternalInput"
        ).ap()
        for i in range(NSEG)
    ]
    w2_d = [
        nc.dram_tensor(
            f"w2_s{i}", [128, c.DC * c.HC * 128], BF16, kind="ExternalInput"
        ).ap()
        for i in range(NSEG)
    ]
    yt_out = nc.dram_tensor(
        "yt", [128, c.DC * c.TPC], BF16, kind="ExternalOutput"
    ).ap()

    with ExitStack() as ctx:
        tc = ctx.enter_context(tile.TileContext(nc))

        wpool = ctx.enter_context(tc.tile_pool(name="w13", bufs=1))
        w2pool = ctx.enter_context(tc.tile_pool(name="w2s", bufs=3))
        xpool = ctx.enter_context(tc.tile_pool(name="xg", bufs=2))
        spool = ctx.enter_context(tc.tile_pool(name="sall", bufs=2))
        ypool = ctx.enter_context(tc.tile_pool(name="yt", bufs=2))
        apool = ctx.enter_context(tc.tile_pool(name="act", bufs=2))
        psum = ctx.enter_context(tc.tile_pool(name="psum", bufs=2, space="PSUM"))

        def emit_g2(si, s_t, goff, tb, preloaded=None):
            yt_t = ypool.tile([128, c.DC, tb], BF16, tag="yt")
            for d in range(c.DC):
                if preloaded is not None and d < len(preloaded):
                    w2d = preloaded[d]
                else:
                    w2d = w2pool.tile([128, c.HC * 128], BF16, tag="w2d")
                    nc.sync.dma_start(
                        out=w2d[:],
                        in_=w2_d[si][:, d * c.HC * 128 : (d + 1) * c.HC * 128],
                    )
                p2 = psum.tile([128, tb], F32, tag="p2")
                for hc in range(c.HC):
                    nc.tensor.matmul(
                        out=p2[:],
                        lhsT=w2d[:, hc * 128 : (hc + 1) * 128],
                        rhs=s_t[:, hc, :],
                        start=(hc == 0),
                        stop=(hc == c.HC - 1),
                    )
                nc.vector.tensor_copy(out=yt_t[:, d, :], in_=p2[:])
                # write out per d-tile so the final DMA isn't on the tail
                nc.sync.dma_start(
                    out=yt_out[:, goff * c.DC + d * tb : goff * c.DC + (d + 1) * tb],
                    in_=yt_t[:, d, :],
                )

        PRE_W13 = 3  # segment-0 w13 tiles pre-issued before compute starts

        def dma_w13(eng, si, hc, t):
            # two half-DMAs (w1 | w3): finer quanta smooth the cold-start
            # stagger, and the first matmuls only need the w1 half
            base = hc * 2 * c.DC * 128
            mid = base + c.DC * 128
            eng.dma_start(out=t[:, : c.DC * 128], in_=w13_d[si][:, base:mid])
            eng.dma_start(
                out=t[:, c.DC * 128 :],
                in_=w13_d[si][:, mid : base + 2 * c.DC * 128],
            )

        def dma_xin(eng, goff, tb, xg):
            half = c.DC // 2
            eng.dma_start(
                out=xg[:, :half, :],
                in_=xin[:, goff * c.DC : goff * c.DC + half * tb],
            )
            eng.dma_start(
                out=xg[:, half:, :],
                in_=xin[:, goff * c.DC + half * tb : (goff + tb) * c.DC],
            )

        def emit_g13_hc(xg, s_t, tb, hc, w13sb, after_silu=None):
            w1h = w13sb[hc][:, : c.DC * 128]
            w3h = w13sb[hc][:, c.DC * 128 :]
            p1 = psum.tile([128, tb], F32, tag="p1")
            p3 = psum.tile([128, tb], F32, tag="p3")
            for k in range(c.DC):
                nc.tensor.matmul(
                    out=p1[:],
                    lhsT=w1h[:, k * 128 : (k + 1) * 128],
                    rhs=xg[:, k, :],
                    start=(k == 0),
                    stop=(k == c.DC - 1),
                )
            for k in range(c.DC):
                nc.tensor.matmul(
                    out=p3[:],
                    lhsT=w3h[:, k * 128 : (k + 1) * 128],
                    rhs=xg[:, k, :],
                    start=(k == 0),
                    stop=(k == c.DC - 1),
                )
            silu_t = apool.tile([128, tb], F32, tag="silu")
            nc.scalar.activation(
                silu_t[:], p1[:], mybir.ActivationFunctionType.Silu
            )
            if after_silu is not None:
                after_silu()
            nc.vector.tensor_tensor(
                out=s_t[:, hc, :], in0=silu_t[:], in1=p3[:],
                op=mybir.AluOpType.mult,
            )

        goff = 0
        for si in range(NSEG):
            # (re)load this segment's w1/w3 into resident SBUF tiles. For
            # si>0 the WAR deps on the previous segment's last reads stagger
            # these DMAs to the PE's per-hc cadence. For si==0 there is no
            # such gating, and the DMA rings process all transfers queued at
            # t=0 breadth-first, so a flood delays even the first tile by
            # the whole load time; instead pre-issue only the first few
            # tiles and release the rest from the scalar (silu) stream,
            # which advances at the PE's pace.
            w13sb = [
                wpool.tile(
                    [128, 2 * c.DC * 128], BF16, tag=f"w13_{hc}",
                    name=f"w13sb_s{si}_{hc}",
                )
                for hc in range(c.HC)
            ]
            for hc in range(PRE_W13 if si == 0 else c.HC):
                dma_w13(nc.sync, si, hc, w13sb[hc])

            pending = []
            xg_pre = {}
            if si == 0:
                # pre-allocate blocks 0/1 tiles so block 1's xin DMA can be
                # released from block 0's scalar stream (t=0 flood control)
                for bi in (0, 1):
                    xg_pre[bi] = xpool.tile(
                        [128, c.DC, seg_blocks[0][bi]], BF16, tag="xg",
                        name=f"xg_pre{bi}",
                    )
                xg_b1_todo = (
                    goff + seg_blocks[0][0], seg_blocks[0][1], xg_pre[1]
                )
            for bi, tb in enumerate(seg_blocks[si]):
                first = si == 0 and bi == 0
                xg = xg_pre.get(bi) if si == 0 else None
                if xg is None:
                    xg = xpool.tile([128, c.DC, tb], BF16, tag="xg")
                if not (si == 0 and bi == 1):
                    dma_xin(nc.sync, goff, tb, xg)
                s_t = spool.tile([128, c.HC, tb], BF16, tag="s")
                w2_pre = [] if first else None

                def releases(hc, w2_pre=w2_pre, first=first):
                    if not first:
                        return None

                    def go():
                        # scalar-stream-released DMAs, paced by PE progress
                        if hc + PRE_W13 < c.HC:
                            dma_w13(
                                nc.scalar, 0, hc + PRE_W13, w13sb[hc + PRE_W13]
                            )
                        if hc == 10:
                            dma_xin(nc.scalar, *xg_b1_todo)
                        if hc in (c.HC - 3, c.HC - 2, c.HC - 1):
                            w2d = w2pool.tile(
                                [128, c.HC * 128], BF16, tag="w2d"
                            )
                            d = hc - (c.HC - 3)
                            nc.scalar.dma_start(
                                out=w2d[:],
                                in_=w2_d[0][
                                    :, d * c.HC * 128 : (d + 1) * c.HC * 128
                                ],
                            )
                            w2_pre.append(w2d)

                    return go

                for hc in range(c.HC):
                    emit_g13_hc(xg, s_t, tb, hc, w13sb, after_silu=releases(hc))
                pending.append((si, s_t, goff, tb, w2_pre))
                if len(pending) > 1:
                    emit_g2(*pending.pop(0))
                goff += tb
            # flush at segment end so the next segment's w13 DMAs hide
            # behind the trailing GEMM2s instead of stalling the PE
            for p in pending:
                emit_g2(*p)

    nc.compile()
    return nc


# ---------------- host-side routing, dispatch and combine ----------------


def _plan_bins(needs, seg_units, n_cores):
    """Assign each expert's unit count to (core, segment) slots.

    `needs` and `seg_units` are in UNIT-token units. Returns
    (slot_expert[core][seg] = expert id, expert_slots[e] = [(core, seg,
    size_units), ...]) or None if infeasible. Search is slack-pruned
    (total overshoot across experts is bounded by spare capacity) with
    memoized failure states."""
    sizes = sorted(set(seg_units), reverse=True)
    ns = len(sizes)
    avail0 = tuple(list(seg_units).count(s) * n_cores for s in sizes)
    order = sorted(range(len(needs)), key=lambda e: -needs[e])
    slack0 = sum(seg_units) * n_cores - sum(needs)
    if slack0 < 0:
        return None
    seen = set()
    assign = {}

    def options(need, av, slack):
        res = []

        def rec(i, used, total):
            if total >= need:
                if total - need <= slack:
                    res.append(
                        (total - need, tuple(used) + (0,) * (ns - len(used)))
                    )
                return
            if i == ns:
                return
            for n in range(min(av[i], -(-need // sizes[i])), -1, -1):
                rec(i + 1, used + [n], total + n * sizes[i])

        rec(0, [], 0)
        res.sort()
        return res

    def bt(i, av, slack):
        if i == len(order):
            return True
        key = (i, av, slack)
        if key in seen:
            return False
        e = order[i]
        for waste, used in options(needs[e], av, slack):
            assign[e] = used
            if bt(
                i + 1,
                tuple(av[j] - used[j] for j in range(ns)),
                slack - waste,
            ):
                return True
            del assign[e]
        seen.add(key)
        return False

    if not bt(0, avail0, slack0):
        return None

    # materialize slots: slot list in (core, seg) order with capacities
    slot_expert = [[None] * len(seg_units) for _ in range(n_cores)]
    free = {s: [] for s in sizes}
    for core in range(n_cores):
        for seg, s in enumerate(seg_units):
            free[s].append((core, seg))
    expert_slots = {}
    for e in order:
        sl = []
        for j, s in enumerate(sizes):
            for _ in range(assign[e][j]):
                core, seg = free[s].pop(0)
                slot_expert[core][seg] = e
                sl.append((core, seg, s))
        expert_slots[e] = sl
    # unused slots -> expert 0 with zero tokens
    for core in range(n_cores):
        for seg in range(len(seg_units)):
            if slot_expert[core][seg] is None:
                slot_expert[core][seg] = 0
    return slot_expert, expert_slots


def _host_route(cfg, x, router_w):
    c = cfg
    xf = np.ascontiguousarray(
        np.asarray(x, dtype=np.float32).reshape(c.S, c.D)
    )
    logits = xf @ np.asarray(router_w, dtype=np.float32)  # [S, E] fp32
    idx = np.argsort(-logits, axis=1, kind="stable")[:, :2]  # ties: low idx
    v = np.take_along_axis(logits, idx, axis=1)
    v = v - v.max(axis=1, keepdims=True)
    ev = np.exp(v)
    gates = ev / ev.sum(axis=1, keepdims=True)  # [S, 2] fp32
    return xf, idx, gates


def _prep(cfg, xf, idx):
    """Build per-core xin arrays + slot bookkeeping from routing decisions."""
    c = cfg
    NSEG = len(c.SEG_UNITS)
    counts = np.bincount(idx.reshape(-1), minlength=c.E)
    needs = [-(-int(cn) // UNIT) for cn in counts]

    plan = _plan_bins(needs, c.SEG_UNITS, c.E)
    if plan is None:
        raise RuntimeError(f"bin planning failed for counts {counts}")
    slot_expert, expert_slots = plan

    # expert pair lists: (token, rank) sorted by token then rank
    pair_t = {}
    pair_r = {}
    for e in range(c.E):
        t_arr, r_arr = np.nonzero(idx == e)
        pair_t[e] = t_arr.astype(np.int64)
        pair_r[e] = r_arr.astype(np.int64)

    seg_off = np.cumsum([0] + [UNIT * u for u in c.SEG_UNITS])[:-1]
    # token slot table per core and position map (token, rank) -> global row
    tok_core = np.full((c.E, c.TPC), -1, dtype=np.int64)
    pos = np.full((c.S, 2), -1, dtype=np.int64)
    for e in range(c.E):
        off = 0
        for (core, seg, s) in expert_slots[e]:
            cap = UNIT * s
            n = min(cap, len(pair_t[e]) - off)
            if n <= 0:
                continue
            rows = seg_off[seg] + np.arange(n)
            tok_core[core, rows] = pair_t[e][off : off + n]
            pos[pair_t[e][off : off + n], pair_r[e][off : off + n]] = (
                core * c.TPC + rows
            )
            off += n
        assert off >= len(pair_t[e]), f"expert {e} tokens unassigned"
    assert (pos >= 0).all(), "unassigned (token, rank) pair"

    # per-core xin in block layout [128, (b, k, t)]
    xbf = xf.astype(ml_dtypes.bfloat16)
    blocks = []
    goff = 0
    for st in c.SEG_UNITS:
        for tb in _blocks_of(st):
            blocks.append((goff, tb))
            goff += tb
    xins = []
    for core in range(c.E):
        toks = tok_core[core]
        g = xbf[np.clip(toks, 0, None)]
        g[toks < 0] = 0
        parts = []
        for (boff, tb) in blocks:
            blk = g[boff : boff + tb]  # [tb, D]
            parts.append(
                np.ascontiguousarray(
                    blk.reshape(tb, c.DC, 128).transpose(2, 1, 0)
                ).reshape(128, c.DC * tb)
            )
        xins.append(np.ascontiguousarray(np.concatenate(parts, axis=1)))
    return slot_expert, pos, xins, blocks


def _prep_weights(cfg, w1, w3, w2):
    c = cfg
    W13, W2 = [], []
    for e in range(c.E):
        w1e = np.asarray(w1[e], dtype=np.float32).astype(ml_dtypes.bfloat16)
        w3e = np.asarray(w3[e], dtype=np.float32).astype(ml_dtypes.bfloat16)
        w2e = np.asarray(w2[e], dtype=np.float32).astype(ml_dtypes.bfloat16)
        w1te = (
            w1e.reshape(c.DC, 128, c.HC, 128)
            .transpose(1, 2, 0, 3)
            .reshape(128, c.HC * c.DC * 128)
        )
        w3te = (
            w3e.reshape(c.DC, 128, c.HC, 128)
            .transpose(1, 2, 0, 3)
            .reshape(128, c.HC * c.DC * 128)
        )
        w13te = np.ascontiguousarray(
            np.stack([w1te, w3te], axis=1)
            .reshape(128, 2, c.HC, c.DC * 128)
            .transpose(0, 2, 1, 3)
            .reshape(128, c.HC * 2 * c.DC * 128)
        )
        w2te = np.ascontiguousarray(
            w2e.reshape(c.HC, 128, c.DC, 128)
            .transpose(1, 2, 0, 3)
            .reshape(128, c.DC * c.HC * 128)
        )
        W13.append(w13te)
        W2.append(w2te)
    return W13, W2


def _combine(cfg, results, pos, gates, blocks):
    c = cfg
    ys = []
    for core in range(c.E):
        yt = np.asarray(results[core]["yt"])  # [128, DC*TPC] bf16
        parts = []
        col = 0
        for (boff, tb) in blocks:
            blk = yt[:, col : col + c.DC * tb].reshape(128, c.DC, tb)
            parts.append(
                blk.transpose(2, 1, 0).reshape(tb, c.D).astype(np.float32)
            )
            col += c.DC * tb
        ys.append(np.concatenate(parts, axis=0))
    y_all = np.concatenate(ys, axis=0)  # [E*TPC, D] fp32 ungated
    out = (
        gates[:, 0:1] * y_all[pos[:, 0]] + gates[:, 1:2] * y_all[pos[:, 1]]
    )
    return out


_PROGRAM_CACHE = {}


def _get_program(cfg: Cfg):
    if cfg not in _PROGRAM_CACHE:
        _PROGRAM_CACHE[cfg] = build_program(cfg, debug=False)
    return _PROGRAM_CACHE[cfg]


def _install_trace_shims():
    """The agent image's antenv lacks axon_hooks; recreate it from the
    boot package's ctypes NTFF driver so trace=True works under axon."""
    import types

    try:
        import antenv
        from antenv.axon_hooks import get_axon_ntff_profile_hook  # noqa: F401

        have = True
    except ImportError:
        have = False
    if not have:
        try:
            import antenv
            from trn_agent_boot.trn_boot import _ntff_profile_via_ctypes

            hook = _ntff_profile_via_ctypes("/opt/axon/libaxon_pjrt.so")
            mod = types.ModuleType("antenv.axon_hooks")
            mod.get_axon_ntff_profile_hook = lambda: hook
            mod.set_axon_ntff_profile_hook = lambda h: None
            sys.modules["antenv.axon_hooks"] = mod
            antenv.axon_hooks = mod
        except Exception as e:
            print(f"trace shim failed ({e}); tracing disabled")
            return False
    from concourse import bass_utils as _bu

    _orig_upload = _bu.upload_artifacts

    def _safe_upload(tmpdir):
        try:
            return _orig_upload(tmpdir)
        except Exception as e:
            return f"upload-skipped({e.__class__.__name__}):{tmpdir}"

    _bu.upload_artifacts = _safe_upload
    return True


def run(cfg: Cfg, x, router_w, w1, w3, w2, trace=False):
    from concourse.bass_utils import run_bass_kernel_spmd

    if trace and not _install_trace_shims():
        trace = False

    c = cfg
    xf, idx, gates = _host_route(c, x, router_w)
    counts = np.bincount(idx.reshape(-1), minlength=c.E)
    needs = [-(-int(cn) // UNIT) for cn in counts]
    # grow segments if the planned capacity is infeasible (recompiles)
    for _ in range(64):
        if _plan_bins(needs, c.SEG_UNITS, c.E) is not None:
            break
        st = list(c.SEG_UNITS)
        st[0] += 1
        c = Cfg(SEG_UNITS=tuple(st))
    else:
        raise RuntimeError(f"no feasible bin plan for counts {counts}")

    slot_expert, pos, xins, blocks = _prep(c, xf, idx)
    W13, W2 = _prep_weights(c, w1, w3, w2)

    in_maps = []
    for core in range(c.E):
        m = {"xin": xins[core]}
        for seg in range(len(c.SEG_UNITS)):
            e = slot_expert[core][seg]
            m[f"w13_s{seg}"] = W13[e]
            m[f"w2_s{seg}"] = W2[e]
        in_maps.append(m)

    nc = _get_program(c)
    res = run_bass_kernel_spmd(
        nc, in_maps, core_ids=list(range(c.E)), trace=trace
    )
    out = _combine(c, res.results, pos, gates, blocks)
    return out, res


def kernel(x, router_w, w1, w3, w2):
    out, _ = run(REAL, x, router_w, w1, w3, w2, trace=False)
    return out.reshape(np.asarray(x).shape).astype(np.float32)


if __name__ == "__main__":
    nc = build_program(REAL)
    print("built ok")


# revision 23
# speedup vs baseline: 1.0597x; 1.0044x over previous
"""Trainium2 Bass kernel for a top-2 MoE layer (8 experts), expert-parallel
across 8 NeuronCores.

Math (per reference):
    logits = x @ router_w                    # [S, E] fp32
    top2 vals/idx; gates = softmax(top2)     # [S, 2]
    out = sum_e gate_e * (silu(x@w1[e]) * (x@w3[e])) @ w2[e]

Distribution strategy (expert-parallel, host-side dispatch): the router GEMM
is 0.05% of total FLOPs, so the host computes it exactly in fp32 and
dispatches (token, expert) pairs to the 8 cores. Each core's program is a
pure streaming SwiGLU FFN over a fixed schedule of 4 weight "segments"
(48+46+21+14 = 129 units of 32 tokens = 4128 token slots per core); the
host bin-packs each expert's token list into the 32 (core, segment) slots
so every real (token, expert) pair is computed exactly once (1.1% padding).
Expert weights for each (core, segment) are uploaded per slot; w1/w3 stay
SBUF-resident for a whole segment while w2 streams per output tile. Gates
are applied on the host during the final gather-combine (y is linear in
w2's output, so the device returns ungated per-pair outputs in a transposed
[d, token] layout and the host does out[t] = g0*y[pos0[t]] + g1*y[pos1[t]]).

The device pipeline per block (up to 512 tokens): DMA x-block -> 22x
(8 matmuls w1 + 8 matmuls w3 -> PSUM; Silu on Scalar; mult on Vector ->
s_all bf16) -> GEMM2 (8 output tiles x 22 matmuls, w2 streamed) -> DMA out.
GEMM2 for block b is emitted after GEMM1/3 of block b+1 within a segment so
the PE never waits on the Vector engine; it is flushed at segment end so the
next segment's w1/w3 DMAs overlap the last two GEMM2s.
"""

import os
import sys

for _p in ("/opt/trn_rl_repo",):
    if _p not in sys.path and os.path.isdir(_p):
        sys.path.insert(0, _p)

from contextlib import ExitStack
from dataclasses import dataclass

import numpy as np
import ml_dtypes

from concourse import bacc, bass, mybir
import concourse.tile as tile

F32 = mybir.dt.float32
BF16 = mybir.dt.bfloat16
UNIT = 32  # dispatch granularity in tokens


@dataclass(frozen=True)
class Cfg:
    S: int = 16384      # tokens
    D: int = 1024       # d_model
    H: int = 2816       # hidden
    E: int = 8          # experts == n_cores
    SEG_UNITS: tuple = (48, 46, 21, 14)  # 32-token units per weight segment

    @property
    def DC(self):
        return self.D // 128

    @property
    def HC(self):
        return self.H // 128

    @property
    def TPC(self):
        return UNIT * sum(self.SEG_UNITS)  # token slots per core


REAL = Cfg()


BMAX = 512 // UNIT  # max units per matmul block (PSUM bank = 512 fp32)


def _even_split(n, cap):
    nb = -(-n // cap)
    base, rem = divmod(n, nb)
    return [base + (1 if i < rem else 0) for i in range(nb)]


def _blocks_of(nunits):
    """Split a segment of `nunits` UNIT-token units into matmul blocks of
    at most 512 tokens. Prefer a max-size first block (so the PE outpaces
    the segment's weight-load DMA at cold start) unless that leaves a runt
    block; blocks under ~290 tokens lose matmul efficiency."""
    if nunits <= BMAX:
        return [UNIT * nunits]
    first_big = [BMAX] + _even_split(nunits - BMAX, BMAX)
    if min(first_big) * UNIT >= 288:
        return [UNIT * s for s in first_big]
    return [UNIT * s for s in _even_split(nunits, BMAX)]


def build_program(cfg: Cfg, debug: bool = False):
    c = cfg
    NSEG = len(c.SEG_UNITS)
    seg_blocks = [_blocks_of(u) for u in c.SEG_UNITS]

    nc = bacc.Bacc(
        "TRN2", target_bir_lowering=False, debug=debug, num_devices=c.E
    )

    xin = nc.dram_tensor(
        "xin", [128, c.DC * c.TPC], BF16, kind="ExternalInput"
    ).ap()
    w13_d = [
        nc.dram_tensor(
            f"w13_s{i}", [128, c.HC * 2 * c.DC * 128], BF16, kind="ExternalInput"
        ).ap()
        for i in range(NSEG)
    ]
    w2_d = [
        nc.dram_tensor(
            f"w2_s{i}", [128, c.DC * c.HC * 128], BF16, kind="ExternalInput"
        ).ap()
        for i in range(NSEG)
    ]
    yt_out = nc.dram_tensor(
        "yt", [128, c.DC * c.TPC], BF16, kind="ExternalOutput"
    ).ap()

    with ExitStack() as ctx:
        tc = ctx.enter_context(tile.TileContext(nc))

        wpool = ctx.enter_context(tc.tile_pool(name="w13", bufs=1))
        w2pool = ctx.enter_context(tc.tile_pool(name="w2s", bufs=3))
        xpool = ctx.enter_context(tc.tile_pool(name="xg", bufs=2))
        spool = ctx.enter_context(tc.tile_pool(name="sall", bufs=2))
        ypool = ctx.enter_context(tc.tile_pool(name="yt", bufs=2))
        apool = ctx.enter_context(tc.tile_pool(name="act", bufs=2))
        psum = ctx.enter_context(tc.tile_pool(name="psum", bufs=2, space="PSUM"))

        def emit_g2(si, s_t, goff, tb, preloaded=None):
            yt_t = ypool.tile([128, c.DC, tb], BF16, tag="yt")
            for d in range(c.DC):
                if preloaded is not None and d < len(preloaded):
                    w2d = preloaded[d]
                else:
                    w2d = w2pool.tile([128, c.HC * 128], BF16, tag="w2d")
                    nc.sync.dma_start(
                        out=w2d[:],
                        in_=w2_d[si][:, d * c.HC * 128 : (d + 1) * c.HC * 128],
                    )
                p2 = psum.tile([128, tb], F32, tag="p2")
                for hc in range(c.HC):
                    nc.tensor.matmul(
                        out=p2[:],
                        lhsT=w2d[:, hc * 128 : (hc + 1) * 128],
                        rhs=s_t[:, hc, :],
                        start=(hc == 0),
                        stop=(hc == c.HC - 1),
                    )
                nc.vector.tensor_copy(out=yt_t[:, d, :], in_=p2[:])
                # write out per d-tile so the final DMA isn't on the tail
                nc.sync.dma_start(
                    out=yt_out[:, goff * c.DC + d * tb : goff * c.DC + (d + 1) * tb],
                    in_=yt_t[:, d, :],
                )

        PRE_W13 = 3  # segment-0 w13 tiles pre-issued before compute starts

        def dma_w13(eng, si, hc, t):
            # two half-DMAs (w1 | w3): finer quanta smooth the cold-start
            # stagger, and the first matmuls only need the w1 half
            base = hc * 2 * c.DC * 128
            mid = base + c.DC * 128
            eng.dma_start(out=t[:, : c.DC * 128], in_=w13_d[si][:, base:mid])
            eng.dma_start(
                out=t[:, c.DC * 128 :],
                in_=w13_d[si][:, mid : base + 2 * c.DC * 128],
            )

        def dma_xin(eng, goff, tb, xg):
            half = c.DC // 2
            eng.dma_start(
                out=xg[:, :half, :],
                in_=xin[:, goff * c.DC : goff * c.DC + half * tb],
            )
            eng.dma_start(
                out=xg[:, half:, :],
                in_=xin[:, goff * c.DC + half * tb : (goff + tb) * c.DC],
            )

        def emit_g13_hc(xg, s_t, tb, hc, w13sb, after_silu=None):
            w1h = w13sb[hc][:, : c.DC * 128]
            w3h = w13sb[hc][:, c.DC * 128 :]
            p1 = psum.tile([128, tb], F32, tag="p1")
            p3 = psum.tile([128, tb], F32, tag="p3")
            for k in range(c.DC):
                nc.tensor.matmul(
                    out=p1[:],
                    lhsT=w1h[:, k * 128 : (k + 1) * 128],
                    rhs=xg[:, k, :],
                    start=(k == 0),
                    stop=(k == c.DC - 1),
                )
            for k in range(c.DC):
                nc.tensor.matmul(
                    out=p3[:],
                    lhsT=w3h[:, k * 128 : (k + 1) * 128],
                    rhs=xg[:, k, :],
                    start=(k == 0),
                    stop=(k == c.DC - 1),
                )
            silu_t = apool.tile([128, tb], F32, tag="silu")
            nc.scalar.activation(
                silu_t[:], p1[:], mybir.ActivationFunctionType.Silu
            )
            if after_silu is not None:
                after_silu()
            nc.vector.tensor_tensor(
                out=s_t[:, hc, :], in0=silu_t[:], in1=p3[:],
                op=mybir.AluOpType.mult,
            )

        goff = 0
        for si in range(NSEG):
            # (re)load this segment's w1/w3 into resident SBUF tiles. For
            # si>0 the WAR deps on the previous segment's last reads stagger
            # these DMAs to the PE's per-hc cadence. For si==0 there is no
            # such gating, and the DMA rings process all transfers queued at
            # t=0 breadth-first, so a flood delays even the first tile by
            # the whole load time; instead pre-issue only the first few
            # tiles and release the rest from the scalar (silu) stream,
            # which advances at the PE's pace.
            w13sb = [
                wpool.tile(
                    [128, 2 * c.DC * 128], BF16, tag=f"w13_{hc}",
                    name=f"w13sb_s{si}_{hc}",
                )
                for hc in range(c.HC)
            ]
            for hc in range(PRE_W13 if si == 0 else c.HC):
                dma_w13(nc.sync, si, hc, w13sb[hc])

            pending = []
            xg_pre = {}
            if si == 0:
                # pre-allocate blocks 0/1 tiles so block 1's xin DMA can be
                # released from block 0's scalar stream (t=0 flood control)
                for bi in (0, 1):
                    xg_pre[bi] = xpool.tile(
                        [128, c.DC, seg_blocks[0][bi]], BF16, tag="xg",
                        name=f"xg_pre{bi}",
                    )
                xg_b1_todo = (
                    goff + seg_blocks[0][0], seg_blocks[0][1], xg_pre[1]
                )
            for bi, tb in enumerate(seg_blocks[si]):
                first = si == 0 and bi == 0
                xg = xg_pre.get(bi) if si == 0 else None
                if xg is None:
                    xg = xpool.tile([128, c.DC, tb], BF16, tag="xg")
                if not (si == 0 and bi == 1):
                    dma_xin(nc.sync, goff, tb, xg)
                s_t = spool.tile([128, c.HC, tb], BF16, tag="s")
                w2_pre = [] if first else None

                def releases(hc, w2_pre=w2_pre, first=first):
                    if not first:
                        return None

                    def go():
                        # scalar-stream-released DMAs, paced by PE progress
                        if hc + PRE_W13 < c.HC:
                            dma_w13(
                                nc.scalar, 0, hc + PRE_W13, w13sb[hc + PRE_W13]
                            )
                        if hc == 10:
                            dma_xin(nc.scalar, *xg_b1_todo)
                        if hc in (c.HC - 3, c.HC - 2, c.HC - 1):
                            w2d = w2pool.tile(
                                [128, c.HC * 128], BF16, tag="w2d"
                            )
                            d = hc - (c.HC - 3)
                            nc.scalar.dma_start(
                                out=w2d[:],
                                in_=w2_d[0][
                                    :, d * c.HC * 128 : (d + 1) * c.HC * 128
                                ],
                            )
                            w2_pre.append(w2d)

                    return go

                for hc in range(c.HC):
                    emit_g13_hc(xg, s_t, tb, hc, w13sb, after_silu=releases(hc))
                pending.append((si, s_t, goff, tb, w2_pre))
                if len(pending) > 1:
                    emit_g2(*pending.pop(0))
                goff += tb
            # flush at segment end so the next segment's w13 DMAs hide
            # behind the trailing GEMM2s instead of stalling the PE
            for p in pending:
                emit_g2(*p)

    nc.compile()
    return nc


# ---------------- host-side routing, dispatch and combine ----------------


def _plan_bins(needs, seg_units, n_cores):
    """Assign each expert's unit count to (core, segment) slots.

    `needs` and `seg_units` are in UNIT-token units. Returns
    (slot_expert[core][seg] = expert id, expert_slots[e] = [(core, seg,
    size_units), ...]) or None if infeasible. Search is slack-pruned
    (total overshoot across experts is bounded by spare capacity) with
    memoized failure states."""
    sizes = sorted(set(seg_units), reverse=True)
    ns = len(sizes)
    avail0 = tuple(list(seg_units).count(s) * n_cores for s in sizes)
    order = sorted(range(len(needs)), key=lambda e: -needs[e])
    slack0 = sum(seg_units) * n_cores - sum(needs)
    if slack0 < 0:
        return None
    seen = set()
    assign = {}

    def options(need, av, slack):
        res = []

        def rec(i, used, total):
            if total >= need:
                if total - need <= slack:
                    res.append(
                        (total - need, tuple(used) + (0,) * (ns - len(used)))
                    )
                return
            if i == ns:
                return
            for n in range(min(av[i], -(-need // sizes[i])), -1, -1):
                rec(i + 1, used + [n], total + n * sizes[i])

        rec(0, [], 0)
        res.sort()
        return res

    def bt(i, av, slack):
        if i == len(order):
            return True
        key = (i, av, slack)
        if key in seen:
            return False
        e = order[i]
        for waste, used in options(needs[e], av, slack):
            assign[e] = used
            if bt(
                i + 1,
                tuple(av[j] - used[j] for j in range(ns)),
                slack - waste,
            ):
                return True
            del assign[e]
        seen.add(key)
        return False

    if not bt(0, avail0, slack0):
        return None

    # materialize slots: slot list in (core, seg) order with capacities
    slot_expert = [[None] * len(seg_units) for _ in range(n_cores)]
    free = {s: [] for s in sizes}
    for core in range(n_cores):
        for seg, s in enumerate(seg_units):
            free[s].append((core, seg))
    expert_slots = {}
    for e in order:
        sl = []
        for j, s in enumerate(sizes):
            for _ in range(assign[e][j]):
                core, seg = free[s].pop(0)
                slot_expert[core][seg] = e
                sl.append((core, seg, s))
        expert_slots[e] = sl
    # unused slots -> expert 0 with zero tokens
    for core in range(n_cores):
        for seg in range(len(seg_units)):
            if slot_expert[core][seg] is None:
                slot_expert[core][seg] = 0
    return slot_expert, expert_slots


def _host_route(cfg, x, router_w):
    c = cfg
    xf = np.ascontiguousarray(
        np.asarray(x, dtype=np.float32).reshape(c.S, c.D)
    )
    logits = xf @ np.asarray(router_w, dtype=np.float32)  # [S, E] fp32
    idx = np.argsort(-logits, axis=1, kind="stable")[:, :2]  # ties: low idx
    v = np.take_along_axis(logits, idx, axis=1)
    v = v - v.max(axis=1, keepdims=True)
    ev = np.exp(v)
    gates = ev / ev.sum(axis=1, keepdims=True)  # [S, 2] fp32
    return xf, idx, gates


def _prep(cfg, xf, idx):
    """Build per-core xin arrays + slot bookkeeping from routing decisions."""
    c = cfg
    NSEG = len(c.SEG_UNITS)
    counts = np.bincount(idx.reshape(-1), minlength=c.E)
    needs = [-(-int(cn) // UNIT) for cn in counts]

    plan = _plan_bins(needs, c.SEG_UNITS, c.E)
    if plan is None:
        raise RuntimeError(f"bin planning failed for counts {counts}")
    slot_expert, expert_slots = plan

    # expert pair lists: (token, rank) sorted by token then rank
    pair_t = {}
    pair_r = {}
    for e in range(c.E):
        t_arr, r_arr = np.nonzero(idx == e)
        pair_t[e] = t_arr.astype(np.int64)
        pair_r[e] = r_arr.astype(np.int64)

    seg_off = np.cumsum([0] + [UNIT * u for u in c.SEG_UNITS])[:-1]
    # token slot table per core and position map (token, rank) -> global row
    tok_core = np.full((c.E, c.TPC), -1, dtype=np.int64)
    pos = np.full((c.S, 2), -1, dtype=np.int64)
    for e in range(c.E):
        off = 0
        for (core, seg, s) in expert_slots[e]:
            cap = UNIT * s
            n = min(cap, len(pair_t[e]) - off)
            if n <= 0:
                continue
            rows = seg_off[seg] + np.arange(n)
            tok_core[core, rows] = pair_t[e][off : off + n]
            pos[pair_t[e][off : off + n], pair_r[e][off : off + n]] = (
                core * c.TPC + rows
            )
            off += n
        assert off >= len(pair_t[e]), f"expert {e} tokens unassigned"
    assert (pos >= 0).all(), "unassigned (token, rank) pair"

    # per-core xin in block layout [128, (b, k, t)]
    xbf = xf.astype(ml_dtypes.bfloat16)
    blocks = []
    goff = 0
    for st in c.SEG_UNITS:
        for tb in _blocks_of(st):
            blocks.append((goff, tb))
            goff += tb
    xins = []
    for core in range(c.E):
        toks = tok_core[core]
        g = xbf[np.clip(toks, 0, None)]
        g[toks < 0] = 0
        parts = []
        for (boff, tb) in blocks:
            blk = g[boff : boff + tb]  # [tb, D]
            parts.append(
                np.ascontiguousarray(
                    blk.reshape(tb, c.DC, 128).transpose(2, 1, 0)
                ).reshape(128, c.DC * tb)
            )
        xins.append(np.ascontiguousarray(np.concatenate(parts, axis=1)))
    return slot_expert, pos, xins, blocks


def _prep_weights(cfg, w1, w3, w2):
    c = cfg
    W13, W2 = [], []
    for e in range(c.E):
        w1e = np.asarray(w1[e], dtype=np.float32).astype(ml_dtypes.bfloat16)
        w3e = np.asarray(w3[e], dtype=np.float32).astype(ml_dtypes.bfloat16)
        w2e = np.asarray(w2[e], dtype=np.float32).astype(ml_dtypes.bfloat16)
        w1te = (
            w1e.reshape(c.DC, 128, c.HC, 128)
            .transpose(1, 2, 0, 3)
            .reshape(128, c.HC * c.DC * 128)
        )
        w3te = (
            w3e.reshape(c.DC, 128, c.HC, 128)
            .transpose(1, 2, 0, 3)
            .reshape(128, c.HC * c.DC * 128)
        )
        w13te = np.ascontiguousarray(
            np.stack([w1te, w3te], axis=1)
            .reshape(128, 2, c.HC, c.DC * 128)
            .transpose(0, 2, 1, 3)
            .reshape(128, c.HC * 2 * c.DC * 128)
        )
        w2te = np.ascontiguousarray(
            w2e.reshape(c.HC, 128, c.DC, 128)
            .transpose(1, 2, 0, 3)
            .reshape(128, c.DC * c.HC * 128)
        )
        W13.append(w13te)
        W2.append(w2te)
    return W13, W2


def _combine(cfg, results, pos, gates, blocks):
    c = cfg
    ys = []
    for core in range(c.E):
        yt = np.asarray(results[core]["yt"])  # [128, DC*TPC] bf16
        parts = []
        col = 0
        for (boff, tb) in blocks:
            blk = yt[:, col : col + c.DC * tb].reshape(128, c.DC, tb)
            parts.append(
                blk.transpose(2, 1, 0).reshape(tb, c.D).astype(np.float32)
            )
            col += c.DC * tb
        ys.append(np.concatenate(parts, axis=0))
    y_all = np.concatenate(ys, axis=0)  # [E*TPC, D] fp32 ungated
    out = (
        gates[:, 0:1] * y_all[pos[:, 0]] + gates[:, 1:2] * y_all[pos[:, 1]]
    )
    return out


_PROGRAM_CACHE = {}


def _get_program(cfg: Cfg):
    if cfg not in _PROGRAM_CACHE:
        _PROGRAM_CACHE[cfg] = build_program(cfg, debug=False)
    return _PROGRAM_CACHE[cfg]


def _install_trace_shims():
    """The agent image's antenv lacks axon_hooks; recreate it from the
    boot package's ctypes NTFF driver so trace=True works under axon."""
    import types

    try:
        import antenv
        from antenv.axon_hooks import get_axon_ntff_profile_hook  # noqa: F401

        have = True
    except ImportError:
        have = False
    if not have:
        try:
            import antenv
            from trn_agent_boot.trn_boot import _ntff_profile_via_ctypes

            hook = _ntff_profile_via_ctypes("/opt/axon/libaxon_pjrt.so")
            mod = types.ModuleType("antenv.axon_hooks")
            mod.get_axon_ntff_profile_hook = lambda: hook
            mod.set_axon_ntff_profile_hook = lambda h: None
            sys.modules["antenv.axon_hooks"] = mod
            antenv.axon_hooks = mod
        except Exception as e:
            print(f"trace shim failed ({e}); tracing disabled")
            return False
    from concourse import bass_utils as _bu

    _orig_upload = _bu.upload_artifacts

    def _safe_upload(tmpdir):
        try:
            return _orig_upload(tmpdir)
        except Exception as e:
            return f"upload-skipped({e.__class__.__name__}):{tmpdir}"

    _bu.upload_artifacts = _safe_upload
    return True


def run(cfg: Cfg, x, router_w, w1, w3, w2, trace=False):
    from concourse.bass_utils import run_bass_kernel_spmd

    if trace and not _install_trace_shims():
        trace = False

    c = cfg
    xf, idx, gates = _host_route(c, x, router_w)
    counts = np.bincount(idx.reshape(-1), minlength=c.E)
    needs = [-(-int(cn) // UNIT) for cn in counts]
    # grow segments if the planned capacity is infeasible (recompiles)
    for _ in range(64):
        if _plan_bins(needs, c.SEG_UNITS, c.E) is not None:
            break
        st = list(c.SEG_UNITS)
        st[0] += 1
        c = Cfg(SEG_UNITS=tuple(st))
    else:
        raise RuntimeError(f"no feasible bin plan for counts {counts}")

    slot_expert, pos, xins, blocks = _prep(c, xf, idx)
    W13, W2 = _prep_weights(c, w1, w3, w2)

    in_maps = []
    for core in range(c.E):
        m = {"xin": xins[core]}
        for seg in range(len(c.SEG_UNITS)):
            e = slot_expert[core][seg]
            m[f"w13_s{seg}"] = W13[e]
            m[f"w2_s{seg}"] = W2[e]
        in_maps.append(m)

    nc = _get_program(c)
    res = run_bass_kernel_spmd(
        nc, in_maps, core_ids=list(range(c.E)), trace=trace
    )
    out = _combine(c, res.results, pos, gates, blocks)
    return out, res


def kernel(x, router_w, w1, w3, w2):
    out, _ = run(REAL, x, router_w, w1, w3, w2, trace=False)
    return out.reshape(np.asarray(x).shape).astype(np.float32)


if __name__ == "__main__":
    nc = build_program(REAL)
    print("built ok")


# revision 24
# speedup vs baseline: 1.0636x; 1.0037x over previous
"""Trainium2 Bass kernel for a top-2 MoE layer (8 experts), expert-parallel
across 8 NeuronCores.

Math (per reference):
    logits = x @ router_w                    # [S, E] fp32
    top2 vals/idx; gates = softmax(top2)     # [S, 2]
    out = sum_e gate_e * (silu(x@w1[e]) * (x@w3[e])) @ w2[e]

Distribution strategy (expert-parallel, host-side dispatch): the router GEMM
is 0.05% of total FLOPs, so the host computes it exactly in fp32 and
dispatches (token, expert) pairs to the 8 cores. Each core's program is a
pure streaming SwiGLU FFN over a fixed schedule of 4 weight "segments"
(48+46+21+14 = 129 units of 32 tokens = 4128 token slots per core); the
host bin-packs each expert's token list into the 32 (core, segment) slots
so every real (token, expert) pair is computed exactly once (1.1% padding).
Expert weights for each (core, segment) are uploaded per slot; w1/w3 stay
SBUF-resident for a whole segment while w2 streams per output tile. Gates
are applied on the host during the final gather-combine (y is linear in
w2's output, so the device returns ungated per-pair outputs in a transposed
[d, token] layout and the host does out[t] = g0*y[pos0[t]] + g1*y[pos1[t]]).

The device pipeline per block (up to 512 tokens): DMA x-block -> 22x
(8 matmuls w1 + 8 matmuls w3 -> PSUM; Silu on Scalar; mult on Vector ->
s_all bf16) -> GEMM2 (8 output tiles x 22 matmuls, w2 streamed) -> DMA out.
GEMM2 for block b is emitted after GEMM1/3 of block b+1 within a segment so
the PE never waits on the Vector engine; it is flushed at segment end so the
next segment's w1/w3 DMAs overlap the last two GEMM2s.
"""

import os
import sys

for _p in ("/opt/trn_rl_repo",):
    if _p not in sys.path and os.path.isdir(_p):
        sys.path.insert(0, _p)

from contextlib import ExitStack
from dataclasses import dataclass

import numpy as np
import ml_dtypes

from concourse import bacc, bass, mybir
import concourse.tile as tile

F32 = mybir.dt.float32
BF16 = mybir.dt.bfloat16
UNIT = 32  # dispatch granularity in tokens


@dataclass(frozen=True)
class Cfg:
    S: int = 16384      # tokens
    D: int = 1024       # d_model
    H: int = 2816       # hidden
    E: int = 8          # experts == n_cores
    SEG_UNITS: tuple = (48, 46, 21, 14)  # 32-token units per weight segment

    @property
    def DC(self):
        return self.D // 128

    @property
    def HC(self):
        return self.H // 128

    @property
    def TPC(self):
        return UNIT * sum(self.SEG_UNITS)  # token slots per core


REAL = Cfg()


BMAX = 512 // UNIT  # max units per matmul block (PSUM bank = 512 fp32)


def _even_split(n, cap):
    nb = -(-n // cap)
    base, rem = divmod(n, nb)
    return [base + (1 if i < rem else 0) for i in range(nb)]


def _blocks_of(nunits):
    """Split a segment of `nunits` UNIT-token units into matmul blocks of
    at most 512 tokens. Prefer a max-size first block (so the PE outpaces
    the segment's weight-load DMA at cold start) unless that leaves a runt
    block; blocks under ~290 tokens lose matmul efficiency."""
    if nunits <= BMAX:
        return [UNIT * nunits]
    first_big = [BMAX] + _even_split(nunits - BMAX, BMAX)
    if min(first_big) * UNIT >= 288:
        return [UNIT * s for s in first_big]
    return [UNIT * s for s in _even_split(nunits, BMAX)]


def build_program(cfg: Cfg, debug: bool = False):
    c = cfg
    NSEG = len(c.SEG_UNITS)
    seg_blocks = [_blocks_of(u) for u in c.SEG_UNITS]

    nc = bacc.Bacc(
        "TRN2", target_bir_lowering=False, debug=debug, num_devices=c.E
    )

    xin = nc.dram_tensor(
        "xin", [128, c.DC * c.TPC], BF16, kind="ExternalInput"
    ).ap()
    w13_d = [
        nc.dram_tensor(
            f"w13_s{i}", [128, c.HC * 2 * c.DC * 128], BF16, kind="ExternalInput"
        ).ap()
        for i in range(NSEG)
    ]
    w2_d = [
        nc.dram_tensor(
            f"w2_s{i}", [128, c.DC * c.HC * 128], BF16, kind="ExternalInput"
        ).ap()
        for i in range(NSEG)
    ]
    yt_out = nc.dram_tensor(
        "yt", [128, c.DC * c.TPC], BF16, kind="ExternalOutput"
    ).ap()

    with ExitStack() as ctx:
        tc = ctx.enter_context(tile.TileContext(nc))

        wpool = ctx.enter_context(tc.tile_pool(name="w13", bufs=1))
        w2pool = ctx.enter_context(tc.tile_pool(name="w2s", bufs=3))
        xpool = ctx.enter_context(tc.tile_pool(name="xg", bufs=2))
        spool = ctx.enter_context(tc.tile_pool(name="sall", bufs=2))
        ypool = ctx.enter_context(tc.tile_pool(name="yt", bufs=2))
        apool = ctx.enter_context(tc.tile_pool(name="act", bufs=2))
        psum = ctx.enter_context(tc.tile_pool(name="psum", bufs=2, space="PSUM"))

        def emit_g2(si, s_t, goff, tb, preloaded=None):
            yt_t = ypool.tile([128, c.DC, tb], BF16, tag="yt")
            for d in range(c.DC):
                if preloaded is not None and d < len(preloaded):
                    w2d = preloaded[d]
                else:
                    w2d = w2pool.tile([128, c.HC * 128], BF16, tag="w2d")
                    nc.sync.dma_start(
                        out=w2d[:],
                        in_=w2_d[si][:, d * c.HC * 128 : (d + 1) * c.HC * 128],
                    )
                p2 = psum.tile([128, tb], F32, tag="p2")
                for hc in range(c.HC):
                    nc.tensor.matmul(
                        out=p2[:],
                        lhsT=w2d[:, hc * 128 : (hc + 1) * 128],
                        rhs=s_t[:, hc, :],
                        start=(hc == 0),
                        stop=(hc == c.HC - 1),
                    )
                nc.vector.tensor_copy(out=yt_t[:, d, :], in_=p2[:])
                # write out per d-tile so the final DMA isn't on the tail
                nc.sync.dma_start(
                    out=yt_out[:, goff * c.DC + d * tb : goff * c.DC + (d + 1) * tb],
                    in_=yt_t[:, d, :],
                )

        PRE_W13 = 3  # segment-0 w13 tiles pre-issued before compute starts

        def dma_w13(eng, si, hc, t):
            # two half-DMAs (w1 | w3): finer quanta smooth the cold-start
            # stagger, and the first matmuls only need the w1 half
            base = hc * 2 * c.DC * 128
            mid = base + c.DC * 128
            eng.dma_start(out=t[:, : c.DC * 128], in_=w13_d[si][:, base:mid])
            eng.dma_start(
                out=t[:, c.DC * 128 :],
                in_=w13_d[si][:, mid : base + 2 * c.DC * 128],
            )

        def dma_xin(eng, goff, tb, xg):
            half = c.DC // 2
            eng.dma_start(
                out=xg[:, :half, :],
                in_=xin[:, goff * c.DC : goff * c.DC + half * tb],
            )
            eng.dma_start(
                out=xg[:, half:, :],
                in_=xin[:, goff * c.DC + half * tb : (goff + tb) * c.DC],
            )

        def emit_g13_hc(xg, s_t, tb, hc, w13sb, after_silu=None):
            w1h = w13sb[hc][:, : c.DC * 128]
            w3h = w13sb[hc][:, c.DC * 128 :]
            p1 = psum.tile([128, tb], F32, tag="p1")
            p3 = psum.tile([128, tb], F32, tag="p3")
            for k in range(c.DC):
                nc.tensor.matmul(
                    out=p1[:],
                    lhsT=w1h[:, k * 128 : (k + 1) * 128],
                    rhs=xg[:, k, :],
                    start=(k == 0),
                    stop=(k == c.DC - 1),
                )
            for k in range(c.DC):
                nc.tensor.matmul(
                    out=p3[:],
                    lhsT=w3h[:, k * 128 : (k + 1) * 128],
                    rhs=xg[:, k, :],
                    start=(k == 0),
                    stop=(k == c.DC - 1),
                )
            silu_t = apool.tile([128, tb], F32, tag="silu")
            nc.scalar.activation(
                silu_t[:], p1[:], mybir.ActivationFunctionType.Silu
            )
            if after_silu is not None:
                after_silu()
            nc.vector.tensor_tensor(
                out=s_t[:, hc, :], in0=silu_t[:], in1=p3[:],
                op=mybir.AluOpType.mult,
            )

        goff = 0
        for si in range(NSEG):
            # (re)load this segment's w1/w3 into resident SBUF tiles. For
            # si>0 the WAR deps on the previous segment's last reads stagger
            # these DMAs to the PE's per-hc cadence. All DMAs issue from the
            # sync queue: its 8-outstanding-transfer slot throttle delivers
            # the halves roughly in issue order at full bandwidth, and the
            # scalar engine stays free to run the Silu chain (each HWDGE
            # issue costs ~600ns of issuing-engine time, so putting these on
            # scalar would delay the first Silu and stall the PE through the
            # p1-PSUM write-after-read chain).
            w13sb = [
                wpool.tile(
                    [128, 2 * c.DC * 128], BF16, tag=f"w13_{hc}",
                    name=f"w13sb_s{si}_{hc}",
                )
                for hc in range(c.HC)
            ]
            for hc in range(PRE_W13 if si == 0 else c.HC):
                dma_w13(nc.sync, si, hc, w13sb[hc])

            pending = []
            xg_pre = {}
            w2_pre0 = None
            if si == 0:
                # issue order at t=0: first tiles + block-0 x, then the
                # remaining tiles, then block-1 x and the first GEMM2's w2
                for bi in (0, 1):
                    xg_pre[bi] = xpool.tile(
                        [128, c.DC, seg_blocks[0][bi]], BF16, tag="xg",
                        name=f"xg_pre{bi}",
                    )
                dma_xin(nc.sync, goff, seg_blocks[0][0], xg_pre[0])
                for hc in range(PRE_W13, c.HC):
                    dma_w13(nc.sync, si, hc, w13sb[hc])
                dma_xin(
                    nc.sync, goff + seg_blocks[0][0], seg_blocks[0][1],
                    xg_pre[1],
                )
                w2_pre0 = []
                for d in range(3):
                    w2d = w2pool.tile(
                        [128, c.HC * 128], BF16, tag="w2d", name=f"w2pre{d}"
                    )
                    nc.sync.dma_start(
                        out=w2d[:],
                        in_=w2_d[0][:, d * c.HC * 128 : (d + 1) * c.HC * 128],
                    )
                    w2_pre0.append(w2d)
            for bi, tb in enumerate(seg_blocks[si]):
                first = si == 0 and bi == 0
                xg = xg_pre.get(bi) if si == 0 else None
                if xg is None:
                    xg = xpool.tile([128, c.DC, tb], BF16, tag="xg")
                    dma_xin(nc.sync, goff, tb, xg)
                s_t = spool.tile([128, c.HC, tb], BF16, tag="s")
                for hc in range(c.HC):
                    emit_g13_hc(xg, s_t, tb, hc, w13sb)
                pending.append((si, s_t, goff, tb, w2_pre0 if first else None))
                if len(pending) > 1:
                    emit_g2(*pending.pop(0))
                goff += tb
            # flush at segment end so the next segment's w13 DMAs hide
            # behind the trailing GEMM2s instead of stalling the PE
            for p in pending:
                emit_g2(*p)

    nc.compile()
    return nc


# ---------------- host-side routing, dispatch and combine ----------------


def _plan_bins(needs, seg_units, n_cores):
    """Assign each expert's unit count to (core, segment) slots.

    `needs` and `seg_units` are in UNIT-token units. Returns
    (slot_expert[core][seg] = expert id, expert_slots[e] = [(core, seg,
    size_units), ...]) or None if infeasible. Search is slack-pruned
    (total overshoot across experts is bounded by spare capacity) with
    memoized failure states."""
    sizes = sorted(set(seg_units), reverse=True)
    ns = len(sizes)
    avail0 = tuple(list(seg_units).count(s) * n_cores for s in sizes)
    order = sorted(range(len(needs)), key=lambda e: -needs[e])
    slack0 = sum(seg_units) * n_cores - sum(needs)
    if slack0 < 0:
        return None
    seen = set()
    assign = {}

    def options(need, av, slack):
        res = []

        def rec(i, used, total):
            if total >= need:
                if total - need <= slack:
                    res.append(
                        (total - need, tuple(used) + (0,) * (ns - len(used)))
                    )
                return
            if i == ns:
                return
            for n in range(min(av[i], -(-need // sizes[i])), -1, -1):
                rec(i + 1, used + [n], total + n * sizes[i])

        rec(0, [], 0)
        res.sort()
        return res

    def bt(i, av, slack):
        if i == len(order):
            return True
        key = (i, av, slack)
        if key in seen:
            return False
        e = order[i]
        for waste, used in options(needs[e], av, slack):
            assign[e] = used
            if bt(
                i + 1,
                tuple(av[j] - used[j] for j in range(ns)),
                slack - waste,
            ):
                return True
            del assign[e]
        seen.add(key)
        return False

    if not bt(0, avail0, slack0):
        return None

    # materialize slots: slot list in (core, seg) order with capacities
    slot_expert = [[None] * len(seg_units) for _ in range(n_cores)]
    free = {s: [] for s in sizes}
    for core in range(n_cores):
        for seg, s in enumerate(seg_units):
            free[s].append((core, seg))
    expert_slots = {}
    for e in order:
        sl = []
        for j, s in enumerate(sizes):
            for _ in range(assign[e][j]):
                core, seg = free[s].pop(0)
                slot_expert[core][seg] = e
                sl.append((core, seg, s))
        expert_slots[e] = sl
    # unused slots -> expert 0 with zero tokens
    for core in range(n_cores):
        for seg in range(len(seg_units)):
            if slot_expert[core][seg] is None:
                slot_expert[core][seg] = 0
    return slot_expert, expert_slots


def _host_route(cfg, x, router_w):
    c = cfg
    xf = np.ascontiguousarray(
        np.asarray(x, dtype=np.float32).reshape(c.S, c.D)
    )
    logits = xf @ np.asarray(router_w, dtype=np.float32)  # [S, E] fp32
    idx = np.argsort(-logits, axis=1, kind="stable")[:, :2]  # ties: low idx
    v = np.take_along_axis(logits, idx, axis=1)
    v = v - v.max(axis=1, keepdims=True)
    ev = np.exp(v)
    gates = ev / ev.sum(axis=1, keepdims=True)  # [S, 2] fp32
    return xf, idx, gates


def _prep(cfg, xf, idx):
    """Build per-core xin arrays + slot bookkeeping from routing decisions."""
    c = cfg
    NSEG = len(c.SEG_UNITS)
    counts = np.bincount(idx.reshape(-1), minlength=c.E)
    needs = [-(-int(cn) // UNIT) for cn in counts]

    plan = _plan_bins(needs, c.SEG_UNITS, c.E)
    if plan is None:
        raise RuntimeError(f"bin planning failed for counts {counts}")
    slot_expert, expert_slots = plan

    # expert pair lists: (token, rank) sorted by token then rank
    pair_t = {}
    pair_r = {}
    for e in range(c.E):
        t_arr, r_arr = np.nonzero(idx == e)
        pair_t[e] = t_arr.astype(np.int64)
        pair_r[e] = r_arr.astype(np.int64)

    seg_off = np.cumsum([0] + [UNIT * u for u in c.SEG_UNITS])[:-1]
    # token slot table per core and position map (token, rank) -> global row
    tok_core = np.full((c.E, c.TPC), -1, dtype=np.int64)
    pos = np.full((c.S, 2), -1, dtype=np.int64)
    for e in range(c.E):
        off = 0
        for (core, seg, s) in expert_slots[e]:
            cap = UNIT * s
            n = min(cap, len(pair_t[e]) - off)
            if n <= 0:
                continue
            rows = seg_off[seg] + np.arange(n)
            tok_core[core, rows] = pair_t[e][off : off + n]
            pos[pair_t[e][off : off + n], pair_r[e][off : off + n]] = (
                core * c.TPC + rows
            )
            off += n
        assert off >= len(pair_t[e]), f"expert {e} tokens unassigned"
    assert (pos >= 0).all(), "unassigned (token, rank) pair"

    # per-core xin in block layout [128, (b, k, t)]
    xbf = xf.astype(ml_dtypes.bfloat16)
    blocks = []
    goff = 0
    for st in c.SEG_UNITS:
        for tb in _blocks_of(st):
            blocks.append((goff, tb))
            goff += tb
    xins = []
    for core in range(c.E):
        toks = tok_core[core]
        g = xbf[np.clip(toks, 0, None)]
        g[toks < 0] = 0
        parts = []
        for (boff, tb) in blocks:
            blk = g[boff : boff + tb]  # [tb, D]
            parts.append(
                np.ascontiguousarray(
                    blk.reshape(tb, c.DC, 128).transpose(2, 1, 0)
                ).reshape(128, c.DC * tb)
            )
        xins.append(np.ascontiguousarray(np.concatenate(parts, axis=1)))
    return slot_expert, pos, xins, blocks


def _prep_weights(cfg, w1, w3, w2):
    c = cfg
    W13, W2 = [], []
    for e in range(c.E):
        w1e = np.asarray(w1[e], dtype=np.float32).astype(ml_dtypes.bfloat16)
        w3e = np.asarray(w3[e], dtype=np.float32).astype(ml_dtypes.bfloat16)
        w2e = np.asarray(w2[e], dtype=np.float32).astype(ml_dtypes.bfloat16)
        w1te = (
            w1e.reshape(c.DC, 128, c.HC, 128)
            .transpose(1, 2, 0, 3)
            .reshape(128, c.HC * c.DC * 128)
        )
        w3te = (
            w3e.reshape(c.DC, 128, c.HC, 128)
            .transpose(1, 2, 0, 3)
            .reshape(128, c.HC * c.DC * 128)
        )
        w13te = np.ascontiguousarray(
            np.stack([w1te, w3te], axis=1)
            .reshape(128, 2, c.HC, c.DC * 128)
            .transpose(0, 2, 1, 3)
            .reshape(128, c.HC * 2 * c.DC * 128)
        )
        w2te = np.ascontiguousarray(
            w2e.reshape(c.HC, 128, c.DC, 128)
            .transpose(1, 2, 0, 3)
            .reshape(128, c.DC * c.HC * 128)
        )
        W13.append(w13te)
        W2.append(w2te)
    return W13, W2


def _combine(cfg, results, pos, gates, blocks):
    c = cfg
    ys = []
    for core in range(c.E):
        yt = np.asarray(results[core]["yt"])  # [128, DC*TPC] bf16
        parts = []
        col = 0
        for (boff, tb) in blocks:
            blk = yt[:, col : col + c.DC * tb].reshape(128, c.DC, tb)
            parts.append(
                blk.transpose(2, 1, 0).reshape(tb, c.D).astype(np.float32)
            )
            col += c.DC * tb
        ys.append(np.concatenate(parts, axis=0))
    y_all = np.concatenate(ys, axis=0)  # [E*TPC, D] fp32 ungated
    out = (
        gates[:, 0:1] * y_all[pos[:, 0]] + gates[:, 1:2] * y_all[pos[:, 1]]
    )
    return out


_PROGRAM_CACHE = {}


def _get_program(cfg: Cfg):
    if cfg not in _PROGRAM_CACHE:
        _PROGRAM_CACHE[cfg] = build_program(cfg, debug=False)
    return _PROGRAM_CACHE[cfg]


def _install_trace_shims():
    """The agent image's antenv lacks axon_hooks; recreate it from the
    boot package's ctypes NTFF driver so trace=True works under axon."""
    import types

    try:
        import antenv
        from antenv.axon_hooks import get_axon_ntff_profile_hook  # noqa: F401

        have = True
    except ImportError:
        have = False
    if not have:
        try:
            import antenv
            from trn_agent_boot.trn_boot import _ntff_profile_via_ctypes

            hook = _ntff_profile_via_ctypes("/opt/axon/libaxon_pjrt.so")
            mod = types.ModuleType("antenv.axon_hooks")
            mod.get_axon_ntff_profile_hook = lambda: hook
            mod.set_axon_ntff_profile_hook = lambda h: None
            sys.modules["antenv.axon_hooks"] = mod
            antenv.axon_hooks = mod
        except Exception as e:
            print(f"trace shim failed ({e}); tracing disabled")
            return False
    from concourse import bass_utils as _bu

    _orig_upload = _bu.upload_artifacts

    def _safe_upload(tmpdir):
        try:
            return _orig_upload(tmpdir)
        except Exception as e:
            return f"upload-skipped({e.__class__.__name__}):{tmpdir}"

    _bu.upload_artifacts = _safe_upload
    return True


def run(cfg: Cfg, x, router_w, w1, w3, w2, trace=False):
    from concourse.bass_utils import run_bass_kernel_spmd

    if trace and not _install_trace_shims():
        trace = False

    c = cfg
    xf, idx, gates = _host_route(c, x, router_w)
    counts = np.bincount(idx.reshape(-1), minlength=c.E)
    needs = [-(-int(cn) // UNIT) for cn in counts]
    # grow segments if the planned capacity is infeasible (recompiles)
    for _ in range(64):
        if _plan_bins(needs, c.SEG_UNITS, c.E) is not None:
            break
        st = list(c.SEG_UNITS)
        st[0] += 1
        c = Cfg(SEG_UNITS=tuple(st))
    else:
        raise RuntimeError(f"no feasible bin plan for counts {counts}")

    slot_expert, pos, xins, blocks = _prep(c, xf, idx)
    W13, W2 = _prep_weights(c, w1, w3, w2)

    in_maps = []
    for core in range(c.E):
        m = {"xin": xins[core]}
        for seg in range(len(c.SEG_UNITS)):
            e = slot_expert[core][seg]
            m[f"w13_s{seg}"] = W13[e]
            m[f"w2_s{seg}"] = W2[e]
        in_maps.append(m)

    nc = _get_program(c)
    res = run_bass_kernel_spmd(
        nc, in_maps, core_ids=list(range(c.E)), trace=trace
    )
    out = _combine(c, res.results, pos, gates, blocks)
    return out, res


def kernel(x, router_w, w1, w3, w2):
    out, _ = run(REAL, x, router_w, w1, w3, w2, trace=False)
    return out.reshape(np.asarray(x).shape).astype(np.float32)


if __name__ == "__main__":
    nc = build_program(REAL)
    print("built ok")


# revision 25
# speedup vs baseline: 1.0661x; 1.0024x over previous
"""Trainium2 Bass kernel for a top-2 MoE layer (8 experts), expert-parallel
across 8 NeuronCores.

Math (per reference):
    logits = x @ router_w                    # [S, E] fp32
    top2 vals/idx; gates = softmax(top2)     # [S, 2]
    out = sum_e gate_e * (silu(x@w1[e]) * (x@w3[e])) @ w2[e]

Distribution strategy (expert-parallel, host-side dispatch): the router GEMM
is 0.05% of total FLOPs, so the host computes it exactly in fp32 and
dispatches (token, expert) pairs to the 8 cores. Each core's program is a
pure streaming SwiGLU FFN over a fixed schedule of 4 weight "segments"
(48+46+21+14 = 129 units of 32 tokens = 4128 token slots per core); the
host bin-packs each expert's token list into the 32 (core, segment) slots
so every real (token, expert) pair is computed exactly once (1.1% padding).
Expert weights for each (core, segment) are uploaded per slot; w1/w3 stay
SBUF-resident for a whole segment while w2 streams per output tile. Gates
are applied on the host during the final gather-combine (y is linear in
w2's output, so the device returns ungated per-pair outputs in a transposed
[d, token] layout and the host does out[t] = g0*y[pos0[t]] + g1*y[pos1[t]]).

The device pipeline per block (up to 512 tokens): DMA x-block -> 22x
(8 matmuls w1 + 8 matmuls w3 -> PSUM; Silu on Scalar; mult on Vector ->
s_all bf16) -> GEMM2 (8 output tiles x 22 matmuls, w2 streamed) -> DMA out.
GEMM2 for block b is emitted after GEMM1/3 of block b+1 within a segment so
the PE never waits on the Vector engine; it is flushed at segment end so the
next segment's w1/w3 DMAs overlap the last two GEMM2s.
"""

import os
import sys

for _p in ("/opt/trn_rl_repo",):
    if _p not in sys.path and os.path.isdir(_p):
        sys.path.insert(0, _p)

from contextlib import ExitStack
from dataclasses import dataclass

import numpy as np
import ml_dtypes

from concourse import bacc, bass, mybir
import concourse.tile as tile

F32 = mybir.dt.float32
BF16 = mybir.dt.bfloat16
UNIT = 32  # dispatch granularity in tokens


@dataclass(frozen=True)
class Cfg:
    S: int = 16384      # tokens
    D: int = 1024       # d_model
    H: int = 2816       # hidden
    E: int = 8          # experts == n_cores
    SEG_UNITS: tuple = (48, 46, 21, 14)  # 32-token units per weight segment

    @property
    def DC(self):
        return self.D // 128

    @property
    def HC(self):
        return self.H // 128

    @property
    def TPC(self):
        return UNIT * sum(self.SEG_UNITS)  # token slots per core


REAL = Cfg()


BMAX = 512 // UNIT  # max units per matmul block (PSUM bank = 512 fp32)


def _even_split(n, cap):
    nb = -(-n // cap)
    base, rem = divmod(n, nb)
    return [base + (1 if i < rem else 0) for i in range(nb)]


def _blocks_of(nunits):
    """Split a segment of `nunits` UNIT-token units into matmul blocks of
    at most 512 tokens. Prefer a max-size first block (so the PE outpaces
    the segment's weight-load DMA at cold start) unless that leaves a runt
    block; blocks under ~290 tokens lose matmul efficiency."""
    if nunits <= BMAX:
        return [UNIT * nunits]
    first_big = [BMAX] + _even_split(nunits - BMAX, BMAX)
    if min(first_big) * UNIT >= 288:
        return [UNIT * s for s in first_big]
    return [UNIT * s for s in _even_split(nunits, BMAX)]


def build_program(cfg: Cfg, debug: bool = False):
    c = cfg
    NSEG = len(c.SEG_UNITS)
    seg_blocks = [_blocks_of(u) for u in c.SEG_UNITS]

    nc = bacc.Bacc(
        "TRN2", target_bir_lowering=False, debug=debug, num_devices=c.E
    )

    xin = nc.dram_tensor(
        "xin", [128, c.DC * c.TPC], BF16, kind="ExternalInput"
    ).ap()
    w13_d = [
        nc.dram_tensor(
            f"w13_s{i}", [128, c.HC * 2 * c.DC * 128], BF16, kind="ExternalInput"
        ).ap()
        for i in range(NSEG)
    ]
    w2_d = [
        nc.dram_tensor(
            f"w2_s{i}", [128, c.DC * c.HC * 128], BF16, kind="ExternalInput"
        ).ap()
        for i in range(NSEG)
    ]
    yt_out = nc.dram_tensor(
        "yt", [128, c.DC * c.TPC], BF16, kind="ExternalOutput"
    ).ap()

    with ExitStack() as ctx:
        tc = ctx.enter_context(tile.TileContext(nc))

        wpool = ctx.enter_context(tc.tile_pool(name="w13", bufs=1))
        w2pool = ctx.enter_context(tc.tile_pool(name="w2s", bufs=3))
        xpool = ctx.enter_context(tc.tile_pool(name="xg", bufs=2))
        spool = ctx.enter_context(tc.tile_pool(name="sall", bufs=2))
        ypool = ctx.enter_context(tc.tile_pool(name="yt", bufs=2))
        apool = ctx.enter_context(tc.tile_pool(name="act", bufs=2))
        psum = ctx.enter_context(tc.tile_pool(name="psum", bufs=2, space="PSUM"))

        def emit_g2(si, s_t, goff, tb, preloaded=None):
            yt_t = ypool.tile([128, c.DC, tb], BF16, tag="yt")
            for d in range(c.DC):
                if preloaded is not None and d < len(preloaded):
                    w2d = preloaded[d]
                else:
                    w2d = w2pool.tile([128, c.HC * 128], BF16, tag="w2d")
                    nc.sync.dma_start(
                        out=w2d[:],
                        in_=w2_d[si][:, d * c.HC * 128 : (d + 1) * c.HC * 128],
                    )
                p2 = psum.tile([128, tb], F32, tag="p2")
                for hc in range(c.HC):
                    nc.tensor.matmul(
                        out=p2[:],
                        lhsT=w2d[:, hc * 128 : (hc + 1) * 128],
                        rhs=s_t[:, hc, :],
                        start=(hc == 0),
                        stop=(hc == c.HC - 1),
                    )
                nc.vector.tensor_copy(out=yt_t[:, d, :], in_=p2[:])
                # write out per d-tile so the final DMA isn't on the tail
                nc.sync.dma_start(
                    out=yt_out[:, goff * c.DC + d * tb : goff * c.DC + (d + 1) * tb],
                    in_=yt_t[:, d, :],
                )

        PRE_W13 = 3  # segment-0 w13 tiles pre-issued before compute starts

        def dma_w13(eng, si, hc, t):
            # two half-DMAs (w1 | w3): finer quanta smooth the cold-start
            # stagger, and the first matmuls only need the w1 half
            base = hc * 2 * c.DC * 128
            mid = base + c.DC * 128
            eng.dma_start(out=t[:, : c.DC * 128], in_=w13_d[si][:, base:mid])
            eng.dma_start(
                out=t[:, c.DC * 128 :],
                in_=w13_d[si][:, mid : base + 2 * c.DC * 128],
            )

        def dma_xin(eng, goff, tb, xg):
            half = c.DC // 2
            eng.dma_start(
                out=xg[:, :half, :],
                in_=xin[:, goff * c.DC : goff * c.DC + half * tb],
            )
            eng.dma_start(
                out=xg[:, half:, :],
                in_=xin[:, goff * c.DC + half * tb : (goff + tb) * c.DC],
            )

        def emit_g13_hc(xg, s_t, tb, hc, w13sb, after_silu=None):
            w1h = w13sb[hc][:, : c.DC * 128]
            w3h = w13sb[hc][:, c.DC * 128 :]
            p1 = psum.tile([128, tb], F32, tag="p1")
            p3 = psum.tile([128, tb], F32, tag="p3")
            for k in range(c.DC):
                nc.tensor.matmul(
                    out=p1[:],
                    lhsT=w1h[:, k * 128 : (k + 1) * 128],
                    rhs=xg[:, k, :],
                    start=(k == 0),
                    stop=(k == c.DC - 1),
                )
            for k in range(c.DC):
                nc.tensor.matmul(
                    out=p3[:],
                    lhsT=w3h[:, k * 128 : (k + 1) * 128],
                    rhs=xg[:, k, :],
                    start=(k == 0),
                    stop=(k == c.DC - 1),
                )
            silu_t = apool.tile([128, tb], F32, tag="silu")
            nc.scalar.activation(
                silu_t[:], p1[:], mybir.ActivationFunctionType.Silu
            )
            if after_silu is not None:
                after_silu()
            nc.vector.tensor_tensor(
                out=s_t[:, hc, :], in0=silu_t[:], in1=p3[:],
                op=mybir.AluOpType.mult,
            )

        goff = 0
        for si in range(NSEG):
            # (re)load this segment's w1/w3 into resident SBUF tiles. For
            # si>0 the WAR deps on the previous segment's last reads stagger
            # these DMAs to the PE's per-hc cadence. All DMAs issue from the
            # sync queue: its 8-outstanding-transfer slot throttle delivers
            # the halves roughly in issue order at full bandwidth, and the
            # scalar engine stays free to run the Silu chain (each HWDGE
            # issue costs ~600ns of issuing-engine time, so putting these on
            # scalar would delay the first Silu and stall the PE through the
            # p1-PSUM write-after-read chain).
            w13sb = [
                wpool.tile(
                    [128, 2 * c.DC * 128], BF16, tag=f"w13_{hc}",
                    name=f"w13sb_s{si}_{hc}",
                )
                for hc in range(c.HC)
            ]
            if si > 0:
                for hc in range(c.HC):
                    dma_w13(nc.sync, si, hc, w13sb[hc])

            pending = []
            xg_pre = {}
            w2_pre0 = None
            if si == 0:
                # issue order at t=0: the first matmul's inputs (x half 1 +
                # w1 half of tile 0) grab the first queue slots, then the
                # rest of block-0 x and the remaining tiles, then block-1 x
                # and the first GEMM2's w2
                for bi in (0, 1):
                    xg_pre[bi] = xpool.tile(
                        [128, c.DC, seg_blocks[0][bi]], BF16, tag="xg",
                        name=f"xg_pre{bi}",
                    )
                tb0 = seg_blocks[0][0]
                half = c.DC // 2
                nc.sync.dma_start(
                    out=xg_pre[0][:, :half, :],
                    in_=xin[:, goff * c.DC : goff * c.DC + half * tb0],
                )
                nc.sync.dma_start(
                    out=w13sb[0][:, : c.DC * 128],
                    in_=w13_d[0][:, : c.DC * 128],
                )
                nc.sync.dma_start(
                    out=xg_pre[0][:, half:, :],
                    in_=xin[:, goff * c.DC + half * tb0 : (goff + tb0) * c.DC],
                )
                nc.sync.dma_start(
                    out=w13sb[0][:, c.DC * 128 :],
                    in_=w13_d[0][:, c.DC * 128 : 2 * c.DC * 128],
                )
                for hc in range(1, c.HC):
                    dma_w13(nc.sync, si, hc, w13sb[hc])
                dma_xin(
                    nc.sync, goff + seg_blocks[0][0], seg_blocks[0][1],
                    xg_pre[1],
                )
                w2_pre0 = []
                for d in range(3):
                    w2d = w2pool.tile(
                        [128, c.HC * 128], BF16, tag="w2d", name=f"w2pre{d}"
                    )
                    nc.sync.dma_start(
                        out=w2d[:],
                        in_=w2_d[0][:, d * c.HC * 128 : (d + 1) * c.HC * 128],
                    )
                    w2_pre0.append(w2d)
            for bi, tb in enumerate(seg_blocks[si]):
                first = si == 0 and bi == 0
                xg = xg_pre.get(bi) if si == 0 else None
                if xg is None:
                    xg = xpool.tile([128, c.DC, tb], BF16, tag="xg")
                    dma_xin(nc.sync, goff, tb, xg)
                s_t = spool.tile([128, c.HC, tb], BF16, tag="s")
                for hc in range(c.HC):
                    emit_g13_hc(xg, s_t, tb, hc, w13sb)
                pending.append((si, s_t, goff, tb, w2_pre0 if first else None))
                if len(pending) > 1:
                    emit_g2(*pending.pop(0))
                goff += tb
            # flush at segment end so the next segment's w13 DMAs hide
            # behind the trailing GEMM2s instead of stalling the PE
            for p in pending:
                emit_g2(*p)

    nc.compile()
    return nc


# ---------------- host-side routing, dispatch and combine ----------------


def _plan_bins(needs, seg_units, n_cores):
    """Assign each expert's unit count to (core, segment) slots.

    `needs` and `seg_units` are in UNIT-token units. Returns
    (slot_expert[core][seg] = expert id, expert_slots[e] = [(core, seg,
    size_units), ...]) or None if infeasible. Search is slack-pruned
    (total overshoot across experts is bounded by spare capacity) with
    memoized failure states."""
    sizes = sorted(set(seg_units), reverse=True)
    ns = len(sizes)
    avail0 = tuple(list(seg_units).count(s) * n_cores for s in sizes)
    order = sorted(range(len(needs)), key=lambda e: -needs[e])
    slack0 = sum(seg_units) * n_cores - sum(needs)
    if slack0 < 0:
        return None
    seen = set()
    assign = {}

    def options(need, av, slack):
        res = []

        def rec(i, used, total):
            if total >= need:
                if total - need <= slack:
                    res.append(
                        (total - need, tuple(used) + (0,) * (ns - len(used)))
                    )
                return
            if i == ns:
                return
            for n in range(min(av[i], -(-need // sizes[i])), -1, -1):
                rec(i + 1, used + [n], total + n * sizes[i])

        rec(0, [], 0)
        res.sort()
        return res

    def bt(i, av, slack):
        if i == len(order):
            return True
        key = (i, av, slack)
        if key in seen:
            return False
        e = order[i]
        for waste, used in options(needs[e], av, slack):
            assign[e] = used
            if bt(
                i + 1,
                tuple(av[j] - used[j] for j in range(ns)),
                slack - waste,
            ):
                return True
            del assign[e]
        seen.add(key)
        return False

    if not bt(0, avail0, slack0):
        return None

    # materialize slots: slot list in (core, seg) order with capacities
    slot_expert = [[None] * len(seg_units) for _ in range(n_cores)]
    free = {s: [] for s in sizes}
    for core in range(n_cores):
        for seg, s in enumerate(seg_units):
            free[s].append((core, seg))
    expert_slots = {}
    for e in order:
        sl = []
        for j, s in enumerate(sizes):
            for _ in range(assign[e][j]):
                core, seg = free[s].pop(0)
                slot_expert[core][seg] = e
                sl.append((core, seg, s))
        expert_slots[e] = sl
    # unused slots -> expert 0 with zero tokens
    for core in range(n_cores):
        for seg in range(len(seg_units)):
            if slot_expert[core][seg] is None:
                slot_expert[core][seg] = 0
    return slot_expert, expert_slots


def _host_route(cfg, x, router_w):
    c = cfg
    xf = np.ascontiguousarray(
        np.asarray(x, dtype=np.float32).reshape(c.S, c.D)
    )
    logits = xf @ np.asarray(router_w, dtype=np.float32)  # [S, E] fp32
    idx = np.argsort(-logits, axis=1, kind="stable")[:, :2]  # ties: low idx
    v = np.take_along_axis(logits, idx, axis=1)
    v = v - v.max(axis=1, keepdims=True)
    ev = np.exp(v)
    gates = ev / ev.sum(axis=1, keepdims=True)  # [S, 2] fp32
    return xf, idx, gates


def _prep(cfg, xf, idx):
    """Build per-core xin arrays + slot bookkeeping from routing decisions."""
    c = cfg
    NSEG = len(c.SEG_UNITS)
    counts = np.bincount(idx.reshape(-1), minlength=c.E)
    needs = [-(-int(cn) // UNIT) for cn in counts]

    plan = _plan_bins(needs, c.SEG_UNITS, c.E)
    if plan is None:
        raise RuntimeError(f"bin planning failed for counts {counts}")
    slot_expert, expert_slots = plan

    # expert pair lists: (token, rank) sorted by token then rank
    pair_t = {}
    pair_r = {}
    for e in range(c.E):
        t_arr, r_arr = np.nonzero(idx == e)
        pair_t[e] = t_arr.astype(np.int64)
        pair_r[e] = r_arr.astype(np.int64)

    seg_off = np.cumsum([0] + [UNIT * u for u in c.SEG_UNITS])[:-1]
    # token slot table per core and position map (token, rank) -> global row
    tok_core = np.full((c.E, c.TPC), -1, dtype=np.int64)
    pos = np.full((c.S, 2), -1, dtype=np.int64)
    for e in range(c.E):
        off = 0
        for (core, seg, s) in expert_slots[e]:
            cap = UNIT * s
            n = min(cap, len(pair_t[e]) - off)
            if n <= 0:
                continue
            rows = seg_off[seg] + np.arange(n)
            tok_core[core, rows] = pair_t[e][off : off + n]
            pos[pair_t[e][off : off + n], pair_r[e][off : off + n]] = (
                core * c.TPC + rows
            )
            off += n
        assert off >= len(pair_t[e]), f"expert {e} tokens unassigned"
    assert (pos >= 0).all(), "unassigned (token, rank) pair"

    # per-core xin in block layout [128, (b, k, t)]
    xbf = xf.astype(ml_dtypes.bfloat16)
    blocks = []
    goff = 0
    for st in c.SEG_UNITS:
        for tb in _blocks_of(st):
            blocks.append((goff, tb))
            goff += tb
    xins = []
    for core in range(c.E):
        toks = tok_core[core]
        g = xbf[np.clip(toks, 0, None)]
        g[toks < 0] = 0
        parts = []
        for (boff, tb) in blocks:
            blk = g[boff : boff + tb]  # [tb, D]
            parts.append(
                np.ascontiguousarray(
                    blk.reshape(tb, c.DC, 128).transpose(2, 1, 0)
                ).reshape(128, c.DC * tb)
            )
        xins.append(np.ascontiguousarray(np.concatenate(parts, axis=1)))
    return slot_expert, pos, xins, blocks


def _prep_weights(cfg, w1, w3, w2):
    c = cfg
    W13, W2 = [], []
    for e in range(c.E):
        w1e = np.asarray(w1[e], dtype=np.float32).astype(ml_dtypes.bfloat16)
        w3e = np.asarray(w3[e], dtype=np.float32).astype(ml_dtypes.bfloat16)
        w2e = np.asarray(w2[e], dtype=np.float32).astype(ml_dtypes.bfloat16)
        w1te = (
            w1e.reshape(c.DC, 128, c.HC, 128)
            .transpose(1, 2, 0, 3)
            .reshape(128, c.HC * c.DC * 128)
        )
        w3te = (
            w3e.reshape(c.DC, 128, c.HC, 128)
            .transpose(1, 2, 0, 3)
            .reshape(128, c.HC * c.DC * 128)
        )
        w13te = np.ascontiguousarray(
            np.stack([w1te, w3te], axis=1)
            .reshape(128, 2, c.HC, c.DC * 128)
            .transpose(0, 2, 1, 3)
            .reshape(128, c.HC * 2 * c.DC * 128)
        )
        w2te = np.ascontiguousarray(
            w2e.reshape(c.HC, 128, c.DC, 128)
            .transpose(1, 2, 0, 3)
            .reshape(128, c.DC * c.HC * 128)
        )
        W13.append(w13te)
        W2.append(w2te)
    return W13, W2


def _combine(cfg, results, pos, gates, blocks):
    c = cfg
    ys = []
    for core in range(c.E):
        yt = np.asarray(results[core]["yt"])  # [128, DC*TPC] bf16
        parts = []
        col = 0
        for (boff, tb) in blocks:
            blk = yt[:, col : col + c.DC * tb].reshape(128, c.DC, tb)
            parts.append(
                blk.transpose(2, 1, 0).reshape(tb, c.D).astype(np.float32)
            )
            col += c.DC * tb
        ys.append(np.concatenate(parts, axis=0))
    y_all = np.concatenate(ys, axis=0)  # [E*TPC, D] fp32 ungated
    out = (
        gates[:, 0:1] * y_all[pos[:, 0]] + gates[:, 1:2] * y_all[pos[:, 1]]
    )
    return out


_PROGRAM_CACHE = {}


def _get_program(cfg: Cfg):
    if cfg not in _PROGRAM_CACHE:
        _PROGRAM_CACHE[cfg] = build_program(cfg, debug=False)
    return _PROGRAM_CACHE[cfg]


def _install_trace_shims():
    """The agent image's antenv lacks axon_hooks; recreate it from the
    boot package's ctypes NTFF driver so trace=True works under axon."""
    import types

    try:
        import antenv
        from antenv.axon_hooks import get_axon_ntff_profile_hook  # noqa: F401

        have = True
    except ImportError:
        have = False
    if not have:
        try:
            import antenv
            from trn_agent_boot.trn_boot import _ntff_profile_via_ctypes

            hook = _ntff_profile_via_ctypes("/opt/axon/libaxon_pjrt.so")
            mod = types.ModuleType("antenv.axon_hooks")
            mod.get_axon_ntff_profile_hook = lambda: hook
            mod.set_axon_ntff_profile_hook = lambda h: None
            sys.modules["antenv.axon_hooks"] = mod
            antenv.axon_hooks = mod
        except Exception as e:
            print(f"trace shim failed ({e}); tracing disabled")
            return False
    from concourse import bass_utils as _bu

    _orig_upload = _bu.upload_artifacts

    def _safe_upload(tmpdir):
        try:
            return _orig_upload(tmpdir)
        except Exception as e:
            return f"upload-skipped({e.__class__.__name__}):{tmpdir}"

    _bu.upload_artifacts = _safe_upload
    return True


def run(cfg: Cfg, x, router_w, w1, w3, w2, trace=False):
    from concourse.bass_utils import run_bass_kernel_spmd

    if trace and not _install_trace_shims():
        trace = False

    c = cfg
    xf, idx, gates = _host_route(c, x, router_w)
    counts = np.bincount(idx.reshape(-1), minlength=c.E)
    needs = [-(-int(cn) // UNIT) for cn in counts]
    # grow segments if the planned capacity is infeasible (recompiles)
    for _ in range(64):
        if _plan_bins(needs, c.SEG_UNITS, c.E) is not None:
            break
        st = list(c.SEG_UNITS)
        st[0] += 1
        c = Cfg(SEG_UNITS=tuple(st))
    else:
        raise RuntimeError(f"no feasible bin plan for counts {counts}")

    slot_expert, pos, xins, blocks = _prep(c, xf, idx)
    W13, W2 = _prep_weights(c, w1, w3, w2)

    in_maps = []
    for core in range(c.E):
        m = {"xin": xins[core]}
        for seg in range(len(c.SEG_UNITS)):
            e = slot_expert[core][seg]
            m[f"w13_s{seg}"] = W13[e]
            m[f"w2_s{seg}"] = W2[e]
        in_maps.append(m)

    nc = _get_program(c)
    res = run_bass_kernel_spmd(
        nc, in_maps, core_ids=list(range(c.E)), trace=trace
    )
    out = _combine(c, res.results, pos, gates, blocks)
    return out, res


def kernel(x, router_w, w1, w3, w2):
    out, _ = run(REAL, x, router_w, w1, w3, w2, trace=False)
    return out.reshape(np.asarray(x).shape).astype(np.float32)


if __name__ == "__main__":
    nc = build_program(REAL)
    print("built ok")
